# revision 16
# baseline (speedup 1.0000x reference)
"""Trainium2 Bass kernel for DiagonalS5SSM.

Math (per batch b; the reference's where(valid,...) is elided — valid is a
prefix mask in l and the output is masked by the same prefix, so the frozen
tail states never reach the output):

    it[l, n]  = sum_d x[b, l, d] * bbar[n, d]          (complex)
    s[l, n]   = abar[n] * s[l-1, n] + it[l, n]         (complex scan)
    y[b, l, :] = mask[l] * (Re(s[l] @ c^T) + x[b, l] @ D^T)

The complex scan is decoupled into two real scans via polar form
abar = rho * e^{i theta} (rho < 1, so no dynamic-range blowup):

    v[l] = e^{-i theta l} * it[l]       (elementwise rotation)
    w[l] = rho * w[l-1] + v[l]          (HW tensor_tensor_scan per plane)
    s[l] = e^{+i theta l} * w[l]        (rotation back)

Engine split (coarse-grained; fine-grained splits stall the in-order DVE
queue on cross-engine semaphores): forward rotation + scans + back-rotation
add/sub on DVE; back-rotation MULTIPLIES of pair pj on the otherwise-idle
GpSimd/Pool engine, issued at the very start of iteration pj+1 so a full
pair of DVE work hides their latency (Pool only supports TensorTensor-class
opcodes). The x @ D^T residual is folded into the stage-5 matmul as extra
PE contraction steps against D_weight^T blocks (identity fast path), which
kills the xadd stream, its accumulate-DMAs, and the DVE mask ops — the mask
rides the Scalar engine's activation scale on every PSUM evacuation.

Chunk-skipping schedule: since y is zero for l >= lengths[b], only
ceil(lengths[b]/512) chunks per batch ever matter. The host packs those
(batch, chunk) units ("slots") onto the 8 cores (chains of one batch stay
on one core, consecutive slots; long chains may split across cores with a
single discarded "warmup" slot whose zero-init error is rho^512 ~ e^-32).
Every core runs the same M-slot program; per-core differences live in the
packed inputs: x^T slices, rotation-table slices, row masks, and a per-slot
"gate" that multiplies the previous slot's final scan state into the next
slot's initial value (gate=1 continues a chain, gate=0 starts fresh).

Sharding: the slot packing is data-parallel over batch with optional
sequence splits; no collectives. Matmuls and elementwise rotations run in
bf16; the scan coefficient rho stays fp32; scan state is fp32 in hardware.
x is pre-transposed on the host so the contraction dim lands on partitions.
Slots are processed in pairs, software-pipelined: phase-D (stage-5 matmul +
mask) of the previous pair overlaps the DVE work of the current pair.
"""

import math
import time

import numpy as np
import ml_dtypes

import concourse.bass as bass
import concourse.tile as tile
from concourse import bacc, mybir
from concourse.bass_utils import run_bass_kernel_spmd

B, L, D, N = 16, 2048, 512, 256
NCORES = 8
C = 512                   # chunk (slot) length
NH = N // 128             # n-halves
DK = D // 128             # d-chunks

F32 = mybir.dt.float32
BF16 = mybir.dt.bfloat16
BF16_NP = np.dtype(ml_dtypes.bfloat16)

AluOp = mybir.AluOpType
ACT_COPY = mybir.ActivationFunctionType.Copy


def _bcast_cols(ap: bass.AP, n: int) -> bass.AP:
    """[128, 1] AP -> [128, n] free-broadcast (step-0) AP."""
    return bass.AP(tensor=ap.tensor, offset=ap.offset, ap=[ap.ap[0], [0, n]])


def _sub2(ap_c: bass.AP, stride_elems: int) -> bass.AP:
    """[128, C] AP -> [128, 2, C] view with an outer sub-dim."""
    return bass.AP(tensor=ap_c.tensor, offset=ap_c.offset,
                   ap=[ap_c.ap[0], [stride_elems, 2], ap_c.ap[1]])


# --------------------------------------------------------------------------
# schedule planning (host)
# --------------------------------------------------------------------------

class Slot:
    """One 512-row unit of work. real: (batch, chunk) whose output is kept.
    warmup: (batch, chunk) processed only to seed the next slot's scan
    state. dummy: padding (all-zero inputs, zero mask, gate 0)."""

    def __init__(self, kind, batch=-1, chunk=-1, gate=0.0):
        self.kind = kind          # "real" | "warmup" | "dummy"
        self.batch = batch
        self.chunk = chunk
        self.gate = gate          # multiply prev slot's final state into init

    def __repr__(self):
        return f"{self.kind[0]}{self.batch}.{self.chunk}g{int(self.gate)}"


def plan_schedule(lengths):
    """Pack per-batch chunk chains onto NCORES cores.

    Returns (M, static_cont, cores) where cores is a list of NCORES
    slot-lists, each of length M (even).
    """
    lengths = [int(v) for v in lengths]
    nch = [min((l + C - 1) // C, L // C) for l in lengths]
    # chains: (batch, first_real_chunk, n_real, needs_warmup)
    chains = [(b, 0, n, False) for b, n in enumerate(nch) if n > 0]
    total = sum(c[2] for c in chains)
    M = max((total + NCORES - 1) // NCORES, 1)

    def try_pack(M):
        rem = [M] * NCORES
        bins = [[] for _ in range(NCORES)]
        work = sorted(chains, key=lambda c: -(c[2] + c[3]))
        while work:
            ch = work.pop(0)
            b, c0, n, warm = ch
            size = n + (1 if warm else 0)
            # best-fit: smallest remaining capacity that still fits
            cand = [i for i in range(NCORES) if rem[i] >= size]
            if cand:
                i = min(cand, key=lambda i: rem[i])
                bins[i].append(ch)
                rem[i] -= size
                continue
            # must split: use the core with the largest remaining capacity
            i = max(range(NCORES), key=lambda i: rem[i])
            cap = rem[i]
            take_real = cap - (1 if warm else 0)
            if cap < 1 or take_real < 1 or take_real >= n:
                return None
            bins[i].append((b, c0, take_real, warm))
            rem[i] = 0
            # remainder continues on another core behind a warmup slot
            work.insert(0, (b, c0 + take_real, n - take_real, True))
            work.sort(key=lambda c: -(c[2] + c[3]))
        return bins

    while True:
        bins = try_pack(M)
        if bins is not None:
            break
        M += 1
    if M % 2:
        M += 1

    cores = []
    static_cont = M
    for bin_ in bins:
        bin_ = sorted(bin_, key=lambda ch: -(ch[2] + ch[3]))
        if bin_:
            first_slots = bin_[0][2] + (1 if bin_[0][3] else 0)
        else:
            first_slots = 1
        static_cont = min(static_cont, first_slots - 1, 3)
        slots = []
        for (b, c0, n, warm) in bin_:
            if warm:
                slots.append(Slot("warmup", b, c0 - 1, 0.0))
            for j in range(n):
                slots.append(Slot("real", b, c0 + j,
                                  0.0 if (j == 0 and not warm) else 1.0))
        while len(slots) < M:
            slots.append(Slot("dummy"))
        assert len(slots) == M
        cores.append(slots)

    static_cont = max(static_cont, 0)
    for slots in cores:
        for si in range(1, static_cont + 1):
            assert slots[si].gate == 1.0, (static_cont, slots)
    _validate_schedule(lengths, nch, M, cores)
    return M, static_cont, cores


def _validate_schedule(lengths, nch, M, cores):
    seen = {}
    for ci, slots in enumerate(cores):
        state = None  # (batch, last_done_chunk) after each slot
        for s in slots:
            if s.kind == "dummy":
                assert s.gate == 0.0
                state = None
                continue
            if s.gate == 0.0:
                # fresh start: must begin at chunk 0, or be a warmup slot
                assert s.chunk == 0 or s.kind == "warmup", (ci, s)
            else:
                assert state == (s.batch, s.chunk - 1), (ci, s, state)
            state = (s.batch, s.chunk)
            if s.kind == "real":
                assert seen.setdefault((s.batch, s.chunk), ci) == ci
    for b in range(B):
        for c in range(nch[b]):
            assert (b, c) in seen, f"missing chunk {(b, c)}"


# --------------------------------------------------------------------------
# device program (depends only on M / static_cont / D_weight fast path)
# --------------------------------------------------------------------------

def build_nc(M, static_cont=0, dw_is_eye=True):
    R = M * C                 # rows per core
    RT = R // 128             # 128-row tiles per core
    NP = M // 2               # slot pairs

    nc = bacc.Bacc(
        "TRN2",
        target_bir_lowering=False,
        debug=False,
        enable_asserts=False,
        num_devices=NCORES,
    )

    xt_d = nc.dram_tensor("xt", [D, R], BF16, kind="ExternalInput")
    w1_d = nc.dram_tensor("w1", [128, DK * 2 * NH * 128], BF16, kind="ExternalInput")
    w2_d = nc.dram_tensor("w2", [128, 2 * NH * D], BF16, kind="ExternalInput")
    dwt_d = nc.dram_tensor("dwt", [128, DK * D], BF16, kind="ExternalInput")
    cos_d = nc.dram_tensor("cost", [128, NH * R], BF16, kind="ExternalInput")
    sin_d = nc.dram_tensor("sint", [128, NH * R], BF16, kind="ExternalInput")
    rho_d = nc.dram_tensor("rho", [128, NH], F32, kind="ExternalInput")
    gate_d = nc.dram_tensor("gate", [128, M], F32, kind="ExternalInput")
    mask_d = nc.dram_tensor("maskc", [128, RT], F32, kind="ExternalInput")
    y_d = nc.dram_tensor("y", [R, D], BF16, kind="ExternalOutput")

    with tile.TileContext(nc) as tc:
        with (
            tc.tile_pool(name="consts", bufs=1) as consts,
            tc.tile_pool(name="wplanes", bufs=3) as wplanes,
            tc.tile_pool(name="xtp", bufs=6) as xt_p,
            tc.tile_pool(name="uvp", bufs=16) as uv_p,
            tc.tile_pool(name="pqp", bufs=8) as pq_p,
            tc.tile_pool(name="u2p", bufs=4) as u2_p,
            tc.tile_pool(name="sp", bufs=8) as s_p,
            tc.tile_pool(name="yp", bufs=3) as y_p,
            tc.tile_pool(name="ps_it", bufs=2, space="PSUM") as ps_it,
            tc.tile_pool(name="ps_y", bufs=3, space="PSUM") as ps_y,
            tc.tile_pool(name="ps_warm", bufs=1, space="PSUM") as ps_warm,
        ):
            # HAM warmup: the PE clock-gate opens only after ~3.4us of
            # sustained matmul activity. Run throwaway matmuls on a zeroed
            # tile from t~7us (right after the framework preamble) so the
            # real stage-1 stream starts at 2.4 GHz instead of 1.2.
            warm_z = consts.tile([128, 512], BF16, tag="warmz")
            nc.vector.memset(warm_z[:], 0)
            warm_ps = ps_warm.tile([128, 512], F32, tag="warm", name="warmps")
            for wi in range(10):
                nc.tensor.matmul(warm_ps[:], warm_z[:, 0:128], warm_z[:],
                                 start=True, stop=True, skip_group_check=True)

            def pulse(dep_ap):
                # cheap keep-warm matmul paced by a DVE output: keeps the
                # PE HAM window busy during DVE-bound stretches
                nc.tensor.matmul(warm_ps[:, 0:64], warm_z[:, 0:128],
                                 dep_ap[:, 0:64], start=True, stop=True,
                                 skip_group_check=True)
            # Startup-critical loads (w1, first pair's x^T and tables) are
            # spread across the DMA queues so the first stage-1 matmuls and
            # rotations start as early as possible.
            w1_sb = [consts.tile([128, 2 * NH * 128], BF16, tag=f"w1_{k}",
                                 name=f"w1sb_{k}") for k in range(DK)]

            def w1_load(k, eng):
                eng.dma_start(w1_sb[k][:], w1_d.ap()[:, k * 512:(k + 1) * 512])

            cos_sb = consts.tile([128, NH * R], BF16, tag="cos")
            sin_sb = consts.tile([128, NH * R], BF16, tag="sin")

            def tbl_load(pj, h, eng):
                colsl = slice(h * R + 2 * pj * C, h * R + (2 * pj + 2) * C)
                eng.dma_start(cos_sb[:, colsl], cos_d.ap()[:, colsl])
                eng.dma_start(sin_sb[:, colsl], sin_d.ap()[:, colsl])

            xt_first = []
            for si in range(2):
                t = xt_p.tile([128, DK * C], BF16, tag="xt", name=f"xt_{si}")
                xt_first.append(t)

            # sync queue: first pair's x^T (slot 0 split in two for latency)
            nc.sync.dma_start(
                xt_first[0][:, 0:2 * C].rearrange("p (k c) -> p k c", k=2),
                xt_d.ap()[0:256, 0:C].rearrange("(k p) c -> p k c", p=128))
            nc.sync.dma_start(
                xt_first[0][:, 2 * C:].rearrange("p (k c) -> p k c", k=2),
                xt_d.ap()[256:512, 0:C].rearrange("(k p) c -> p k c", p=128))
            nc.sync.dma_start(
                xt_first[1][:].rearrange("p (k c) -> p k c", k=DK),
                xt_d.ap()[:, C:2 * C].rearrange("(k p) c -> p k c", p=128))
            # scalar queue: w1 chunks then pair-0 tables (half 0)
            w1_load(0, nc.scalar)
            w1_load(1, nc.scalar)
            w1_load(2, nc.scalar)
            tbl_load(0, 0, nc.scalar)
            # remaining startup loads on the scalar/sync queues; the gpsimd
            # queue is now a compute engine and stays clear of DMA work
            rho_sb = consts.tile([128, NH], F32, tag="rho")
            nc.scalar.dma_start(rho_sb[:], rho_d.ap())
            w1_load(3, nc.scalar)
            gate_sb = consts.tile([128, M], F32, tag="gate")
            nc.scalar.dma_start(gate_sb[:], gate_d.ap())
            tbl_load(0, 1, nc.scalar)
            w2_sb = consts.tile([128, 2 * NH * D], BF16, tag="w2")
            nc.sync.dma_start(w2_sb[:], w2_d.ap())
            dwt_sb = consts.tile([128, DK * D], BF16, tag="dwt")
            nc.sync.dma_start(dwt_sb[:], dwt_d.ap())
            for pj in range(1, NP):
                for h in range(NH):
                    tbl_load(pj, h, nc.sync)
            mask_sb = consts.tile([128, RT], F32, tag="maskc")
            nc.sync.dma_start(mask_sb[:], mask_d.ap())


            def emit_phase_d(pj, s_ch, xt_pair, last):
                for sub in range(2):
                    si = 2 * pj + sub
                    xt = xt_pair[sub]
                    ysb = y_p.tile([128, 4 * D], BF16, tag="ysb", name=f"ysb_{si}")
                    for rt2 in range(4):
                        rt = si * 4 + rt2
                        scol = sub * C + rt2 * 128
                        ps = ps_y.tile([128, D], F32, tag="y", name=f"ys_{rt}")
                        first = True
                        for plane in range(2):
                            for half in range(NH):
                                nc.tensor.matmul(
                                    ps[:],
                                    s_ch[plane][half][:, scol:scol + 128],
                                    w2_sb[:, (plane * 2 + half) * D:(plane * 2 + half + 1) * D],
                                    start=first,
                                    stop=False,
                                )
                                first = False
                        # x-residual: ps[l, :] += x[l, :] @ Dw^T via
                        # transpose-style matmuls against Dw^T blocks
                        for k in range(DK):
                            lhsT = xt[:, k * C + rt2 * 128: k * C + rt2 * 128 + 128]
                            if dw_is_eye:
                                nc.tensor.matmul(
                                    ps[:, k * 128:(k + 1) * 128],
                                    lhsT,
                                    dwt_sb[:, k * D + k * 128: k * D + (k + 1) * 128],
                                    start=False, stop=(k == DK - 1),
                                    skip_group_check=True,
                                )
                            else:
                                nc.tensor.matmul(
                                    ps[:], lhsT,
                                    dwt_sb[:, k * D:(k + 1) * D],
                                    start=False, stop=(k == DK - 1),
                                    skip_group_check=True,
                                )
                        ycol = slice(rt2 * D, (rt2 + 1) * D)
                        nc.scalar.activation(
                            ysb[:, ycol], ps[:], ACT_COPY,
                            scale=mask_sb[:, rt:rt + 1],
                        )
                        if rt2 % 2 == 1:
                            rq = slice(si * C + (rt2 - 1) * 128,
                                       si * C + (rt2 + 1) * 128)
                            nc.sync.dma_start(
                                y_d.ap()[rq, :].rearrange("(a p) d -> p a d", p=128),
                                ysb[:, (rt2 - 1) * D:(rt2 + 1) * D].rearrange(
                                    "p (a d) -> p a d", a=2),
                            )

            def emit_back_mults(pj, wpair):
                """Pool-engine multiplies for pair pj's back rotation.
                Returns the p1..p4 product tiles per half."""
                prods = []
                for half in range(NH):
                    tcol = slice(half * R + 2 * pj * C, half * R + (2 * pj + 2) * C)
                    cs = cos_sb[:, tcol].rearrange("p (s c) -> p s c", s=2)
                    sn = sin_sb[:, tcol].rearrange("p (s c) -> p s c", s=2)
                    wre = wpair[0][half][:].rearrange("p (s c) -> p s c", s=2)
                    wim = wpair[1][half][:].rearrange("p (s c) -> p s c", s=2)
                    ps4 = [pq_p.tile([128, 2 * C], BF16, tag="pq",
                                     name=f"p{i}_{pj}_{half}") for i in range(4)]

                    def pv(t):
                        return t[:].rearrange("p (s c) -> p s c", s=2)
                    nc.gpsimd.tensor_tensor(pv(ps4[0]), wre, cs, op=AluOp.mult)
                    nc.gpsimd.tensor_tensor(pv(ps4[1]), wim, sn, op=AluOp.mult)
                    nc.gpsimd.tensor_tensor(pv(ps4[2]), wim, cs, op=AluOp.mult)
                    nc.gpsimd.tensor_tensor(pv(ps4[3]), wre, sn, op=AluOp.mult)
                    prods.append(ps4)
                return prods

            def emit_back_adds(pj, prods):
                s_ch = [[None] * NH for _ in range(2)]
                for half in range(NH):
                    p1, p2, p3, p4 = prods[half]
                    sre = s_p.tile([128, 2 * C], BF16, tag="sch",
                                   name=f"sre_{pj}_{half}")
                    sim = s_p.tile([128, 2 * C], BF16, tag="sch",
                                   name=f"sim_{pj}_{half}")
                    nc.vector.tensor_sub(sre[:], p1[:], p2[:])
                    nc.vector.tensor_add(sim[:], p3[:], p4[:])
                    s_ch[0][half] = sre
                    s_ch[1][half] = sim
                return s_ch

            def emit_back_rot_dve(pj, wpair):
                """Last pair: per-sub DVE back rotation so slot 2pj's
                phase-D matmuls start while slot 2pj+1 is still rotating."""
                s_ch = [[None] * NH for _ in range(2)]
                for half in range(NH):
                    sre = s_p.tile([128, 2 * C], BF16, tag="sch",
                                   name=f"sre_{pj}_{half}")
                    sim = s_p.tile([128, 2 * C], BF16, tag="sch",
                                   name=f"sim_{pj}_{half}")
                    for sub in range(2):
                        sl = slice(sub * C, (sub + 1) * C)
                        si = 2 * pj + sub
                        css = cos_sb[:, half * R + si * C: half * R + (si + 1) * C]
                        sns = sin_sb[:, half * R + si * C: half * R + (si + 1) * C]
                        wres = wpair[0][half][:, sub * C:(sub + 1) * C]
                        wims = wpair[1][half][:, sub * C:(sub + 1) * C]
                        q1 = uv_p.tile([128, C], BF16, tag="uvs", name=f"q1_{pj}_{half}_{sub}")
                        nc.vector.tensor_tensor(q1[:], wres, css, op=AluOp.mult)
                        q2 = uv_p.tile([128, C], BF16, tag="uvs", name=f"q2_{pj}_{half}_{sub}")
                        nc.vector.tensor_tensor(q2[:], wims, sns, op=AluOp.mult)
                        nc.vector.tensor_sub(sre[:, sl], q1[:], q2[:])
                        q3 = uv_p.tile([128, C], BF16, tag="uvs", name=f"q3_{pj}_{half}_{sub}")
                        nc.vector.tensor_tensor(q3[:], wims, css, op=AluOp.mult)
                        q4 = uv_p.tile([128, C], BF16, tag="uvs", name=f"q4_{pj}_{half}_{sub}")
                        nc.vector.tensor_tensor(q4[:], wres, sns, op=AluOp.mult)
                        nc.vector.tensor_add(sim[:, sl], q3[:], q4[:])
                    s_ch[0][half] = sre
                    s_ch[1][half] = sim
                return s_ch

            pending = None
            for pj in range(NP):
                # previous pair's back-rotation multiplies start the Pool
                # queue immediately: a full pair of DVE work hides them
                if pending is not None:
                    ppj, pxt, wprev = pending
                    prods_prev = emit_back_mults(ppj, wprev)
                else:
                    wprev = None
                wcur = [
                    [wplanes.tile([128, 2 * C], BF16, tag=f"wp_{p}_{h}",
                                  name=f"w_{pj}_{p}_{h}") for h in range(NH)]
                    for p in range(2)
                ]

                # ---- stage 1: it = x @ bbar^T (bf16 matmuls) ----
                u_t = [u2_p.tile([128, 2 * NH * C], BF16, tag="uv2",
                                 name=f"u_{pj}_{plane}")
                       for plane in range(2)]
                xt_pair = []
                for sub in range(2):
                    si = 2 * pj + sub
                    dcol = slice(si * C, (si + 1) * C)
                    if pj == 0:
                        xt = xt_first[sub]
                    else:
                        xt = xt_p.tile([128, DK * C], BF16, tag="xt",
                                       name=f"xt_{si}")
                        nc.sync.dma_start(
                            xt[:].rearrange("p (k c) -> p k c", k=DK),
                            xt_d.ap()[:, dcol].rearrange("(k p) c -> p k c", p=128))
                    xt_pair.append(xt)
                    for plane in range(2):
                        ps = ps_it.tile([128, NH * C], F32, tag="it",
                                        name=f"it_{si}_{plane}")
                        for half in range(NH):
                            for k in range(DK):
                                col = (plane * 2 + half) * 128
                                nc.tensor.matmul(
                                    ps[:, half * C:(half + 1) * C],
                                    w1_sb[k][:, col:col + 128],
                                    xt[:, k * C:(k + 1) * C],
                                    start=(k == 0),
                                    stop=(k == DK - 1),
                                )
                        nc.scalar.activation(
                            u_t[plane][:, sub * NH * C:(sub + 1) * NH * C],
                            ps[:], ACT_COPY)

                # ---- forward rotation + scans (DVE) ----
                for half in range(NH):
                    tcol = slice(half * R + 2 * pj * C, half * R + (2 * pj + 2) * C)
                    cs = cos_sb[:, tcol].rearrange("p (s c) -> p s c", s=2)
                    sn = sin_sb[:, tcol].rearrange("p (s c) -> p s c", s=2)
                    ure = _sub2(u_t[0][:, half * C:(half + 1) * C], NH * C)
                    uim = _sub2(u_t[1][:, half * C:(half + 1) * C], NH * C)

                    vre = uv_p.tile([128, 2 * C], BF16, tag="uv", name=f"vre_{pj}_{half}")
                    vim = uv_p.tile([128, 2 * C], BF16, tag="uv", name=f"vim_{pj}_{half}")
                    if pj == 0:
                        # first pair: per-slot rotation so the DVE starts as
                        # soon as the FIRST slot's matmuls land
                        for sub in range(2):
                            sl = slice(sub * C, (sub + 1) * C)
                            usl = slice(sub * NH * C + half * C,
                                        sub * NH * C + (half + 1) * C)
                            csl = cos_sb[:, half * R + sub * C: half * R + (sub + 1) * C]
                            snl = sin_sb[:, half * R + sub * C: half * R + (sub + 1) * C]
                            t1 = uv_p.tile([128, C], BF16, tag="uvs",
                                           name=f"t1_{pj}_{half}_{sub}")
                            nc.vector.tensor_tensor(t1[:], u_t[0][:, usl], csl,
                                                    op=AluOp.mult)
                            t2 = uv_p.tile([128, C], BF16, tag="uvs",
                                           name=f"t2_{pj}_{half}_{sub}")
                            nc.vector.tensor_tensor(t2[:], u_t[1][:, usl], snl,
                                                    op=AluOp.mult)
                            t3 = uv_p.tile([128, C], BF16, tag="uvs",
                                           name=f"t3_{pj}_{half}_{sub}")
                            nc.vector.tensor_tensor(t3[:], u_t[1][:, usl], csl,
                                                    op=AluOp.mult)
                            t4 = uv_p.tile([128, C], BF16, tag="uvs",
                                           name=f"t4_{pj}_{half}_{sub}")
                            nc.vector.tensor_tensor(t4[:], u_t[0][:, usl], snl,
                                                    op=AluOp.mult)
                            nc.vector.tensor_add(vre[:, sl], t1[:], t2[:])
                            nc.vector.tensor_sub(vim[:, sl], t3[:], t4[:])
                    else:
                        def pv(t):
                            return t[:].rearrange("p (s c) -> p s c", s=2)
                        t1 = uv_p.tile([128, 2 * C], BF16, tag="uv", name=f"t1_{pj}_{half}")
                        nc.vector.tensor_tensor(pv(t1), ure, cs, op=AluOp.mult)
                        t2 = uv_p.tile([128, 2 * C], BF16, tag="uv", name=f"t2_{pj}_{half}")
                        nc.vector.tensor_tensor(pv(t2), uim, sn, op=AluOp.mult)
                        nc.vector.tensor_add(vre[:], t1[:], t2[:])
                        t3 = uv_p.tile([128, 2 * C], BF16, tag="uv", name=f"t3_{pj}_{half}")
                        nc.vector.tensor_tensor(pv(t3), uim, cs, op=AluOp.mult)
                        t4 = uv_p.tile([128, 2 * C], BF16, tag="uv", name=f"t4_{pj}_{half}")
                        nc.vector.tensor_tensor(pv(t4), ure, sn, op=AluOp.mult)
                        nc.vector.tensor_sub(vim[:], t3[:], t4[:])

                    # chained scans; slot si's init is gate[si] * (slot
                    # si-1's final state)
                    rho_b = _bcast_cols(rho_sb[:, half:half + 1], C)
                    for plane, vch in ((0, vre), (1, vim)):
                        wp = wcur[plane][half]
                        for sub in range(2):
                            si = 2 * pj + sub
                            scol = slice(sub * C, (sub + 1) * C)
                            if sub == 1:
                                prev_ap = wp[:, C - 1:C]
                            elif pj > 0:
                                prev_ap = wprev[plane][half][:, 2 * C - 1:2 * C]
                            else:
                                prev_ap = None
                            if si == 0:
                                init = 0.0
                            elif si <= static_cont:
                                # schedule guarantees continuation here on
                                # every core: chain directly, no gate
                                init = prev_ap
                            else:
                                g = uv_p.tile([128, 1], F32, tag="g",
                                              name=f"g_{si}_{plane}_{half}")
                                nc.vector.tensor_tensor(
                                    g[:], prev_ap,
                                    gate_sb[:, si:si + 1], op=AluOp.mult)
                                init = g[:, 0:1]
                            nc.vector.tensor_tensor_scan(
                                out=wp[:, scol],
                                data0=rho_b,
                                data1=vch[:, sub * C:(sub + 1) * C],
                                initial=init,
                                op0=AluOp.mult,
                                op1=AluOp.add,
                            )
                            if half == 0 or pj == NP - 1:
                                pulse(wp[:, scol])

                # previous pair: back-rotation adds then phase-D
                if pending is not None:
                    s_prev = emit_back_adds(ppj, prods_prev)
                    emit_phase_d(ppj, s_prev, pxt, last=False)

                pending = (pj, xt_pair, wcur)

            ppj, pxt, wlast = pending
            s_last = emit_back_rot_dve(ppj, wlast)
            emit_phase_d(ppj, s_last, pxt, last=True)
            warm_out = consts.tile([128, 1], F32, tag="warmout")
            nc.vector.tensor_copy(warm_out[:], warm_ps[:, 0:1])

    nc.compile()
    return nc


_NC_CACHE = {}


def _get_nc(key):
    if key not in _NC_CACHE:
        _NC_CACHE[key] = build_nc(*key)
    return _NC_CACHE[key]


# --------------------------------------------------------------------------
# host-side data prep
# --------------------------------------------------------------------------

def _host_prep(lambda_real_log, lambda_imag, log_dt, B_re, B_im, C_re, C_im):
    """Schedule-independent parameter prep: w1, w2, rho, theta."""
    lam_re = -np.exp(np.asarray(lambda_real_log, np.float64))
    lam_im = np.asarray(lambda_imag, np.float64)
    dtv = np.log1p(np.exp(np.float64(log_dt))) + 1e-4
    rho = np.exp(dtv * lam_re)                       # [N]
    theta = dtv * lam_im                             # [N]
    lam = lam_re + 1j * lam_im
    abar = np.exp(dtv * lam)
    bb = ((abar - 1.0) / lam)[:, None] * (
        np.asarray(B_re, np.float64) + 1j * np.asarray(B_im, np.float64)
    )                                                # [N, D] complex
    bb_planes = (np.ascontiguousarray(bb.real), np.ascontiguousarray(bb.imag))

    w1 = np.empty((128, DK * 2 * NH * 128), BF16_NP)
    for k in range(DK):
        for plane in range(2):
            for half in range(NH):
                col = ((k * 2 + plane) * 2 + half) * 128
                w1[:, col:col + 128] = bb_planes[plane][
                    half * 128:(half + 1) * 128, k * 128:(k + 1) * 128
                ].T.astype(np.float32)

    w2 = np.empty((128, 2 * NH * D), BF16_NP)
    c_planes = (np.asarray(C_re, np.float64), -np.asarray(C_im, np.float64))
    for plane in range(2):
        for half in range(NH):
            col = (plane * 2 + half) * D
            w2[:, col:col + D] = c_planes[plane][
                :, half * 128:(half + 1) * 128
            ].T.astype(np.float32)

    rho_in = np.empty((128, NH), np.float32)
    for half in range(NH):
        rho_in[:, half] = rho[half * 128:(half + 1) * 128]

    return w1, w2, rho_in, theta


def _pack_core(slots, x, lengths, theta, M):
    """Per-core packed inputs for one slot list."""
    R = M * C
    RT = R // 128
    xt = np.zeros((D, R), BF16_NP)
    cost = np.empty((128, NH * R), BF16_NP)
    sint = np.empty((128, NH * R), BF16_NP)
    gate = np.zeros((128, M), np.float32)
    maskc = np.zeros((128, RT), np.float32)

    l_idx = np.arange(C, dtype=np.float64)
    for si, s in enumerate(slots):
        cols = slice(si * C, (si + 1) * C)
        if s.kind == "dummy":
            l0 = 0
        else:
            l0 = s.chunk * C
            xs = np.asarray(x[s.batch, l0:l0 + C, :])      # [C, D]
            xt[:, cols] = xs.T.astype(BF16_NP)
            if s.kind == "real":
                ml = np.clip(int(lengths[s.batch]) - l0, 0, C)
                rowmask = (np.arange(C) < ml).astype(np.float32)
                maskc[:, si * 4:(si + 1) * 4] = rowmask.reshape(4, 128).T
        gate[:, si] = s.gate
        for half in range(NH):
            ph = theta[half * 128:(half + 1) * 128, None] * (l0 + l_idx)[None, :]
            tc = slice(half * R + si * C, half * R + (si + 1) * C)
            cost[:, tc] = np.cos(ph).astype(BF16_NP)
            sint[:, tc] = np.sin(ph).astype(BF16_NP)
    return {"xt": xt, "cost": cost, "sint": sint, "gate": gate, "maskc": maskc}


def prepare(x, lengths, lambda_real_log, lambda_imag, log_dt,
            B_re, B_im, C_re, C_im, D_weight):
    x = np.asarray(x, np.float32)
    Dw = np.asarray(D_weight, np.float32)
    dw_is_eye = bool(
        Dw.shape == (D, D) and np.array_equal(Dw, np.eye(D, dtype=np.float32)))

    dwt = np.empty((128, DK * D), BF16_NP)
    for k in range(DK):
        dwt[:, k * D:(k + 1) * D] = Dw.T[k * 128:(k + 1) * 128, :]

    M, static_cont, cores = plan_schedule(np.asarray(lengths))
    w1, w2, rho_in, theta = _host_prep(
        lambda_real_log, lambda_imag, log_dt, B_re, B_im, C_re, C_im)

    in_maps = []
    for slots in cores:
        m = _pack_core(slots, x, lengths, theta, M)
        m.update({"w1": w1, "w2": w2, "rho": rho_in, "dwt": dwt})
        in_maps.append(m)
    return (M, static_cont, dw_is_eye), cores, in_maps


def unpack_output(res, M, cores):
    y = np.zeros((B, L, D), np.float32)
    for ci, slots in enumerate(cores):
        yc = np.asarray(res.results[ci]["y"], dtype=np.float32)  # [R, D]
        for si, s in enumerate(slots):
            if s.kind == "real":
                l0 = s.chunk * C
                y[s.batch, l0:l0 + C, :] = yc[si * C:(si + 1) * C, :]
    return y


def kernel(x, lengths, lambda_real_log, lambda_imag, log_dt, B_re, B_im,
           C_re, C_im, D_weight):
    key, cores, in_maps = prepare(
        x, lengths, lambda_real_log, lambda_imag, log_dt,
        B_re, B_im, C_re, C_im, D_weight)
    M = key[0]
    nc = _get_nc(key)

    last_err = None
    for attempt in range(4):  # device errors are occasionally transient under axon
        try:
            if not _NC_CACHE.get(("warm",) + key):
                # throwaway execution: first run in a fresh process is
                # regularly ~15% slower (cold device caches / power state)
                run_bass_kernel_spmd(nc, in_maps, core_ids=list(range(NCORES)))
                _NC_CACHE[("warm",) + key] = True
            res = run_bass_kernel_spmd(nc, in_maps, core_ids=list(range(NCORES)))
            break
        except Exception as e:  # noqa: BLE001
            last_err = e
            time.sleep(5 * (attempt + 1))
    else:
        raise last_err
    return unpack_output(res, M, cores)


# revision 17
# speedup vs baseline: 1.0432x; 1.0432x over previous
"""Trainium2 Bass kernel for DiagonalS5SSM.

Math (per batch b; the reference's where(valid,...) is elided — valid is a
prefix mask in l and the output is masked by the same prefix, so the frozen
tail states never reach the output):

    it[l, n]  = sum_d x[b, l, d] * bbar[n, d]          (complex)
    s[l, n]   = abar[n] * s[l-1, n] + it[l, n]         (complex scan)
    y[b, l, :] = mask[l] * (Re(s[l] @ c^T) + x[b, l] @ D^T)

The complex scan is decoupled into two real scans via polar form
abar = rho * e^{i theta} (rho < 1, so no dynamic-range blowup):

    v[l] = e^{-i theta l} * it[l]       (elementwise rotation)
    w[l] = rho * w[l-1] + v[l]          (HW tensor_tensor_scan per plane)
    s[l] = e^{+i theta l} * w[l]        (rotation back)

Engine split (coarse-grained; fine-grained splits stall the in-order DVE
queue on cross-engine semaphores): forward rotation + scans + back-rotation
add/sub on DVE; back-rotation MULTIPLIES of pair pj on the otherwise-idle
GpSimd/Pool engine, issued at the very start of iteration pj+1 so a full
pair of DVE work hides their latency (Pool only supports TensorTensor-class
opcodes). The x @ D^T residual is folded into the stage-5 matmul as extra
PE contraction steps against D_weight^T blocks (identity fast path), which
kills the xadd stream, its accumulate-DMAs, and the DVE mask ops — the mask
rides the Scalar engine's activation scale on every PSUM evacuation.

Chunk-skipping schedule: since y is zero for l >= lengths[b], only
ceil(lengths[b]/512) chunks per batch ever matter. The host packs those
(batch, chunk) units ("slots") onto the 8 cores (chains of one batch stay
on one core, consecutive slots; long chains may split across cores with a
single discarded "warmup" slot whose zero-init error is rho^512 ~ e^-32).
Every core runs the same M-slot program; per-core differences live in the
packed inputs: x^T slices, rotation-table slices, row masks, and a per-slot
"gate" that multiplies the previous slot's final scan state into the next
slot's initial value (gate=1 continues a chain, gate=0 starts fresh).

Sharding: the slot packing is data-parallel over batch with optional
sequence splits; no collectives. Matmuls and elementwise rotations run in
bf16; the scan coefficient rho stays fp32; scan state is fp32 in hardware.
x is pre-transposed on the host so the contraction dim lands on partitions.
Slots are processed in pairs, software-pipelined: phase-D (stage-5 matmul +
mask) of the previous pair overlaps the DVE work of the current pair.
"""

import math
import time

import numpy as np
import ml_dtypes

import concourse.bass as bass
import concourse.tile as tile
from concourse import bacc, mybir
from concourse.bass_utils import run_bass_kernel_spmd

B, L, D, N = 16, 2048, 512, 256
NCORES = 8
C = 512                   # chunk (slot) length
NH = N // 128             # n-halves
DK = D // 128             # d-chunks

F32 = mybir.dt.float32
BF16 = mybir.dt.bfloat16
BF16_NP = np.dtype(ml_dtypes.bfloat16)

AluOp = mybir.AluOpType
ACT_COPY = mybir.ActivationFunctionType.Copy


def _bcast_cols(ap: bass.AP, n: int) -> bass.AP:
    """[128, 1] AP -> [128, n] free-broadcast (step-0) AP."""
    return bass.AP(tensor=ap.tensor, offset=ap.offset, ap=[ap.ap[0], [0, n]])


def _sub2(ap_c: bass.AP, stride_elems: int) -> bass.AP:
    """[128, C] AP -> [128, 2, C] view with an outer sub-dim."""
    return bass.AP(tensor=ap_c.tensor, offset=ap_c.offset,
                   ap=[ap_c.ap[0], [stride_elems, 2], ap_c.ap[1]])


# --------------------------------------------------------------------------
# schedule planning (host)
# --------------------------------------------------------------------------

class Slot:
    """One 512-row unit of work. real: (batch, chunk) whose output is kept.
    warmup: (batch, chunk) processed only to seed the next slot's scan
    state. dummy: padding (all-zero inputs, zero mask, gate 0)."""

    def __init__(self, kind, batch=-1, chunk=-1, gate=0.0):
        self.kind = kind          # "real" | "warmup" | "dummy"
        self.batch = batch
        self.chunk = chunk
        self.gate = gate          # multiply prev slot's final state into init

    def __repr__(self):
        return f"{self.kind[0]}{self.batch}.{self.chunk}g{int(self.gate)}"


def plan_schedule(lengths):
    """Pack per-batch chunk chains onto NCORES cores.

    Returns (M, static_cont, cores) where cores is a list of NCORES
    slot-lists, each of length M (even).
    """
    lengths = [int(v) for v in lengths]
    nch = [min((l + C - 1) // C, L // C) for l in lengths]
    # chains: (batch, first_real_chunk, n_real, needs_warmup)
    chains = [(b, 0, n, False) for b, n in enumerate(nch) if n > 0]
    total = sum(c[2] for c in chains)
    M = max((total + NCORES - 1) // NCORES, 1)

    def try_pack(M):
        rem = [M] * NCORES
        bins = [[] for _ in range(NCORES)]
        work = sorted(chains, key=lambda c: -(c[2] + c[3]))
        while work:
            ch = work.pop(0)
            b, c0, n, warm = ch
            size = n + (1 if warm else 0)
            # best-fit: smallest remaining capacity that still fits
            cand = [i for i in range(NCORES) if rem[i] >= size]
            if cand:
                i = min(cand, key=lambda i: rem[i])
                bins[i].append(ch)
                rem[i] -= size
                continue
            # must split: use the core with the largest remaining capacity
            i = max(range(NCORES), key=lambda i: rem[i])
            cap = rem[i]
            take_real = cap - (1 if warm else 0)
            if cap < 1 or take_real < 1 or take_real >= n:
                return None
            bins[i].append((b, c0, take_real, warm))
            rem[i] = 0
            # remainder continues on another core behind a warmup slot
            work.insert(0, (b, c0 + take_real, n - take_real, True))
            work.sort(key=lambda c: -(c[2] + c[3]))
        return bins

    while True:
        bins = try_pack(M)
        if bins is not None:
            break
        M += 1
    if M % 2:
        M += 1

    cores = []
    static_cont = M
    for bin_ in bins:
        bin_ = sorted(bin_, key=lambda ch: -(ch[2] + ch[3]))
        if bin_:
            first_slots = bin_[0][2] + (1 if bin_[0][3] else 0)
        else:
            first_slots = 1
        static_cont = min(static_cont, first_slots - 1, 3)
        slots = []
        for (b, c0, n, warm) in bin_:
            if warm:
                slots.append(Slot("warmup", b, c0 - 1, 0.0))
            for j in range(n):
                slots.append(Slot("real", b, c0 + j,
                                  0.0 if (j == 0 and not warm) else 1.0))
        while len(slots) < M:
            slots.append(Slot("dummy"))
        assert len(slots) == M
        cores.append(slots)

    static_cont = max(static_cont, 0)
    for slots in cores:
        for si in range(1, static_cont + 1):
            assert slots[si].gate == 1.0, (static_cont, slots)
    _validate_schedule(lengths, nch, M, cores)
    return M, static_cont, cores


def _validate_schedule(lengths, nch, M, cores):
    seen = {}
    for ci, slots in enumerate(cores):
        state = None  # (batch, last_done_chunk) after each slot
        for s in slots:
            if s.kind == "dummy":
                assert s.gate == 0.0
                state = None
                continue
            if s.gate == 0.0:
                # fresh start: must begin at chunk 0, or be a warmup slot
                assert s.chunk == 0 or s.kind == "warmup", (ci, s)
            else:
                assert state == (s.batch, s.chunk - 1), (ci, s, state)
            state = (s.batch, s.chunk)
            if s.kind == "real":
                assert seen.setdefault((s.batch, s.chunk), ci) == ci
    for b in range(B):
        for c in range(nch[b]):
            assert (b, c) in seen, f"missing chunk {(b, c)}"


# --------------------------------------------------------------------------
# device program (depends only on M / static_cont / D_weight fast path)
# --------------------------------------------------------------------------

def build_nc(M, static_cont=0, dw_is_eye=True):
    R = M * C                 # rows per core
    RT = R // 128             # 128-row tiles per core
    NP = M // 2               # slot pairs

    nc = bacc.Bacc(
        "TRN2",
        target_bir_lowering=False,
        debug=False,
        enable_asserts=False,
        num_devices=NCORES,
    )

    xt_d = nc.dram_tensor("xt", [D, R], BF16, kind="ExternalInput")
    w1_d = nc.dram_tensor("w1", [128, DK * 2 * NH * 128], BF16, kind="ExternalInput")
    w2_d = nc.dram_tensor("w2", [128, 2 * NH * D], BF16, kind="ExternalInput")
    dwt_d = nc.dram_tensor("dwt", [128, DK * D], BF16, kind="ExternalInput")
    cos_d = nc.dram_tensor("cost", [128, NH * R], BF16, kind="ExternalInput")
    sin_d = nc.dram_tensor("sint", [128, NH * R], BF16, kind="ExternalInput")
    nsin_d = nc.dram_tensor("nsint", [128, NH * R], BF16, kind="ExternalInput")
    rho_d = nc.dram_tensor("rho", [128, NH], F32, kind="ExternalInput")
    gate_d = nc.dram_tensor("gate", [128, M], F32, kind="ExternalInput")
    mask_d = nc.dram_tensor("maskc", [128, RT], F32, kind="ExternalInput")
    y_d = nc.dram_tensor("y", [R, D], BF16, kind="ExternalOutput")

    with tile.TileContext(nc) as tc:
        with (
            tc.tile_pool(name="consts", bufs=1) as consts,
            tc.tile_pool(name="wplanes", bufs=3) as wplanes,
            tc.tile_pool(name="xtp", bufs=6) as xt_p,
            tc.tile_pool(name="uvp", bufs=16) as uv_p,
            tc.tile_pool(name="pqp", bufs=8) as pq_p,
            tc.tile_pool(name="u2p", bufs=4) as u2_p,
            tc.tile_pool(name="sp", bufs=8) as s_p,
            tc.tile_pool(name="yp", bufs=3) as y_p,
            tc.tile_pool(name="ps_it", bufs=2, space="PSUM") as ps_it,
            tc.tile_pool(name="ps_y", bufs=3, space="PSUM") as ps_y,
            tc.tile_pool(name="ps_warm", bufs=1, space="PSUM") as ps_warm,
        ):
            # HAM warmup: the PE clock-gate opens only after ~3.4us of
            # sustained matmul activity. Run throwaway matmuls on a zeroed
            # tile from t~7us (right after the framework preamble) so the
            # real stage-1 stream starts at 2.4 GHz instead of 1.2.
            warm_z = consts.tile([128, 512], BF16, tag="warmz")
            nc.vector.memset(warm_z[:], 0)
            warm_ps = ps_warm.tile([128, 512], F32, tag="warm", name="warmps")
            for wi in range(10):
                nc.tensor.matmul(warm_ps[:], warm_z[:, 0:128], warm_z[:],
                                 start=True, stop=True, skip_group_check=True)

            def pulse(dep_ap):
                # cheap keep-warm matmul paced by a DVE output: keeps the
                # PE HAM window busy during DVE-bound stretches
                nc.tensor.matmul(warm_ps[:, 0:64], warm_z[:, 0:128],
                                 dep_ap[:, 0:64], start=True, stop=True,
                                 skip_group_check=True)
            # Startup-critical loads (w1, first pair's x^T and tables) are
            # spread across the DMA queues so the first stage-1 matmuls and
            # rotations start as early as possible.
            w1_sb = [consts.tile([128, 2 * NH * 128], BF16, tag=f"w1_{k}",
                                 name=f"w1sb_{k}") for k in range(DK)]

            def w1_load(k, eng):
                eng.dma_start(w1_sb[k][:], w1_d.ap()[:, k * 512:(k + 1) * 512])

            cos_sb = consts.tile([128, NH * R], BF16, tag="cos")
            sin_sb = consts.tile([128, NH * R], BF16, tag="sin")
            nsin_sb = consts.tile([128, NH * R], BF16, tag="nsin")

            def tbl_load(pj, h, eng):
                colsl = slice(h * R + 2 * pj * C, h * R + (2 * pj + 2) * C)
                eng.dma_start(cos_sb[:, colsl], cos_d.ap()[:, colsl])
                eng.dma_start(sin_sb[:, colsl], sin_d.ap()[:, colsl])

            xt_first = []
            for si in range(2):
                t = xt_p.tile([128, DK * C], BF16, tag="xt", name=f"xt_{si}")
                xt_first.append(t)

            # sync queue: first pair's x^T (slot 0 split in two for latency)
            nc.sync.dma_start(
                xt_first[0][:, 0:2 * C].rearrange("p (k c) -> p k c", k=2),
                xt_d.ap()[0:256, 0:C].rearrange("(k p) c -> p k c", p=128))
            nc.sync.dma_start(
                xt_first[0][:, 2 * C:].rearrange("p (k c) -> p k c", k=2),
                xt_d.ap()[256:512, 0:C].rearrange("(k p) c -> p k c", p=128))
            nc.sync.dma_start(
                xt_first[1][:].rearrange("p (k c) -> p k c", k=DK),
                xt_d.ap()[:, C:2 * C].rearrange("(k p) c -> p k c", p=128))
            # scalar queue: w1 chunks then pair-0 tables (half 0)
            w1_load(0, nc.scalar)
            w1_load(1, nc.scalar)
            w1_load(2, nc.scalar)
            tbl_load(0, 0, nc.scalar)
            # remaining startup loads on the scalar/sync queues; the gpsimd
            # queue is now a compute engine and stays clear of DMA work
            rho_sb = consts.tile([128, NH], F32, tag="rho")
            nc.scalar.dma_start(rho_sb[:], rho_d.ap())
            w1_load(3, nc.scalar)
            gate_sb = consts.tile([128, M], F32, tag="gate")
            nc.scalar.dma_start(gate_sb[:], gate_d.ap())
            tbl_load(0, 1, nc.scalar)
            w2_sb = consts.tile([128, 2 * NH * D], BF16, tag="w2")
            nc.sync.dma_start(w2_sb[:], w2_d.ap())
            dwt_sb = consts.tile([128, DK * D], BF16, tag="dwt")
            nc.sync.dma_start(dwt_sb[:], dwt_d.ap())
            for pj in range(1, NP):
                for h in range(NH):
                    tbl_load(pj, h, nc.sync)
            mask_sb = consts.tile([128, RT], F32, tag="maskc")
            nc.sync.dma_start(mask_sb[:], mask_d.ap())
            for pj in range(NP):
                for h in range(NH):
                    colsl = slice(h * R + 2 * pj * C, h * R + (2 * pj + 2) * C)
                    nc.sync.dma_start(nsin_sb[:, colsl], nsin_d.ap()[:, colsl])


            def emit_phase_d(pj, s_ch, xt_pair, last):
                for sub in range(2):
                    si = 2 * pj + sub
                    xt = xt_pair[sub]
                    ysb = y_p.tile([128, 4 * D], BF16, tag="ysb", name=f"ysb_{si}")
                    for rt2 in range(4):
                        rt = si * 4 + rt2
                        scol = sub * C + rt2 * 128
                        ps = ps_y.tile([128, D], F32, tag="y", name=f"ys_{rt}")
                        first = True
                        for plane in range(2):
                            for half in range(NH):
                                nc.tensor.matmul(
                                    ps[:],
                                    s_ch[plane][half][:, scol:scol + 128],
                                    w2_sb[:, (plane * 2 + half) * D:(plane * 2 + half + 1) * D],
                                    start=first,
                                    stop=False,
                                )
                                first = False
                        # x-residual: ps[l, :] += x[l, :] @ Dw^T via
                        # transpose-style matmuls against Dw^T blocks
                        for k in range(DK):
                            lhsT = xt[:, k * C + rt2 * 128: k * C + rt2 * 128 + 128]
                            if dw_is_eye:
                                nc.tensor.matmul(
                                    ps[:, k * 128:(k + 1) * 128],
                                    lhsT,
                                    dwt_sb[:, k * D + k * 128: k * D + (k + 1) * 128],
                                    start=False, stop=(k == DK - 1),
                                    skip_group_check=True,
                                )
                            else:
                                nc.tensor.matmul(
                                    ps[:], lhsT,
                                    dwt_sb[:, k * D:(k + 1) * D],
                                    start=False, stop=(k == DK - 1),
                                    skip_group_check=True,
                                )
                        ycol = slice(rt2 * D, (rt2 + 1) * D)
                        nc.scalar.activation(
                            ysb[:, ycol], ps[:], ACT_COPY,
                            scale=mask_sb[:, rt:rt + 1],
                        )
                        if rt2 % 2 == 1:
                            rq = slice(si * C + (rt2 - 1) * 128,
                                       si * C + (rt2 + 1) * 128)
                            nc.sync.dma_start(
                                y_d.ap()[rq, :].rearrange("(a p) d -> p a d", p=128),
                                ysb[:, (rt2 - 1) * D:(rt2 + 1) * D].rearrange(
                                    "p (a d) -> p a d", a=2),
                            )

            def emit_back_rot_dve(pj, wpair):
                """Last pair: per-sub DVE back rotation so slot 2pj's
                phase-D matmuls start while slot 2pj+1 is still rotating."""
                s_ch = [[None] * NH for _ in range(2)]
                for half in range(NH):
                    sre = s_p.tile([128, 2 * C], BF16, tag="sch",
                                   name=f"sre_{pj}_{half}")
                    sim = s_p.tile([128, 2 * C], BF16, tag="sch",
                                   name=f"sim_{pj}_{half}")
                    for sub in range(2):
                        sl = slice(sub * C, (sub + 1) * C)
                        si = 2 * pj + sub
                        css = cos_sb[:, half * R + si * C: half * R + (si + 1) * C]
                        sns = sin_sb[:, half * R + si * C: half * R + (si + 1) * C]
                        wres = wpair[0][half][:, sub * C:(sub + 1) * C]
                        wims = wpair[1][half][:, sub * C:(sub + 1) * C]
                        q1 = uv_p.tile([128, C], BF16, tag="uvs", name=f"q1_{pj}_{half}_{sub}")
                        nc.vector.tensor_tensor(q1[:], wres, css, op=AluOp.mult)
                        q2 = uv_p.tile([128, C], BF16, tag="uvs", name=f"q2_{pj}_{half}_{sub}")
                        nc.vector.tensor_tensor(q2[:], wims, sns, op=AluOp.mult)
                        nc.vector.tensor_sub(sre[:, sl], q1[:], q2[:])
                        q3 = uv_p.tile([128, C], BF16, tag="uvs", name=f"q3_{pj}_{half}_{sub}")
                        nc.vector.tensor_tensor(q3[:], wims, css, op=AluOp.mult)
                        q4 = uv_p.tile([128, C], BF16, tag="uvs", name=f"q4_{pj}_{half}_{sub}")
                        nc.vector.tensor_tensor(q4[:], wres, sns, op=AluOp.mult)
                        nc.vector.tensor_add(sim[:, sl], q3[:], q4[:])
                    s_ch[0][half] = sre
                    s_ch[1][half] = sim
                return s_ch

            pending = None
            for pj in range(NP):
                if pending is not None:
                    ppj, pxt, wprev, s_prev = pending
                else:
                    wprev = None
                wcur = [
                    [wplanes.tile([128, 2 * C], BF16, tag=f"wp_{p}_{h}",
                                  name=f"w_{pj}_{p}_{h}") for h in range(NH)]
                    for p in range(2)
                ]

                # ---- stage 1: it = x @ bbar^T (bf16 matmuls) ----
                u_t = [u2_p.tile([128, 2 * NH * C], BF16, tag="uv2",
                                 name=f"u_{pj}_{plane}")
                       for plane in range(2)]
                xt_pair = []
                for sub in range(2):
                    si = 2 * pj + sub
                    dcol = slice(si * C, (si + 1) * C)
                    if pj == 0:
                        xt = xt_first[sub]
                    else:
                        xt = xt_p.tile([128, DK * C], BF16, tag="xt",
                                       name=f"xt_{si}")
                        nc.sync.dma_start(
                            xt[:].rearrange("p (k c) -> p k c", k=DK),
                            xt_d.ap()[:, dcol].rearrange("(k p) c -> p k c", p=128))
                    xt_pair.append(xt)
                    for plane in range(2):
                        ps = ps_it.tile([128, NH * C], F32, tag="it",
                                        name=f"it_{si}_{plane}")
                        for half in range(NH):
                            for k in range(DK):
                                col = (plane * 2 + half) * 128
                                nc.tensor.matmul(
                                    ps[:, half * C:(half + 1) * C],
                                    w1_sb[k][:, col:col + 128],
                                    xt[:, k * C:(k + 1) * C],
                                    start=(k == 0),
                                    stop=(k == DK - 1),
                                )
                        nc.scalar.activation(
                            u_t[plane][:, sub * NH * C:(sub + 1) * NH * C],
                            ps[:], ACT_COPY)

                # software-pipelined phase-D of the previous pair
                if pending is not None:
                    emit_phase_d(ppj, s_prev, pxt, last=False)

                # ---- forward rotation + scans (DVE) ----
                s_ch = [[None] * NH for _ in range(2)]
                for half in range(NH):
                    tcol = slice(half * R + 2 * pj * C, half * R + (2 * pj + 2) * C)
                    cs = cos_sb[:, tcol].rearrange("p (s c) -> p s c", s=2)
                    sn = sin_sb[:, tcol].rearrange("p (s c) -> p s c", s=2)
                    ure = _sub2(u_t[0][:, half * C:(half + 1) * C], NH * C)
                    uim = _sub2(u_t[1][:, half * C:(half + 1) * C], NH * C)

                    vre = uv_p.tile([128, 2 * C], BF16, tag="uv", name=f"vre_{pj}_{half}")
                    vim = uv_p.tile([128, 2 * C], BF16, tag="uv", name=f"vim_{pj}_{half}")
                    if pj == 0:
                        # first pair: per-slot rotation so the DVE starts as
                        # soon as the FIRST slot's matmuls land
                        for sub in range(2):
                            sl = slice(sub * C, (sub + 1) * C)
                            usl = slice(sub * NH * C + half * C,
                                        sub * NH * C + (half + 1) * C)
                            csl = cos_sb[:, half * R + sub * C: half * R + (sub + 1) * C]
                            snl = sin_sb[:, half * R + sub * C: half * R + (sub + 1) * C]
                            t1 = uv_p.tile([128, C], BF16, tag="uvs",
                                           name=f"t1_{pj}_{half}_{sub}")
                            nc.vector.tensor_tensor(t1[:], u_t[0][:, usl], csl,
                                                    op=AluOp.mult)
                            t2 = uv_p.tile([128, C], BF16, tag="uvs",
                                           name=f"t2_{pj}_{half}_{sub}")
                            nc.vector.tensor_tensor(t2[:], u_t[1][:, usl], snl,
                                                    op=AluOp.mult)
                            t3 = uv_p.tile([128, C], BF16, tag="uvs",
                                           name=f"t3_{pj}_{half}_{sub}")
                            nc.vector.tensor_tensor(t3[:], u_t[1][:, usl], csl,
                                                    op=AluOp.mult)
                            t4 = uv_p.tile([128, C], BF16, tag="uvs",
                                           name=f"t4_{pj}_{half}_{sub}")
                            nc.vector.tensor_tensor(t4[:], u_t[0][:, usl], snl,
                                                    op=AluOp.mult)
                            nc.vector.tensor_add(vre[:, sl], t1[:], t2[:])
                            nc.vector.tensor_sub(vim[:, sl], t3[:], t4[:])
                    else:
                        def pv(t):
                            return t[:].rearrange("p (s c) -> p s c", s=2)
                        t1 = uv_p.tile([128, 2 * C], BF16, tag="uv", name=f"t1_{pj}_{half}")
                        nc.vector.tensor_tensor(pv(t1), ure, cs, op=AluOp.mult)
                        t2 = uv_p.tile([128, 2 * C], BF16, tag="uv", name=f"t2_{pj}_{half}")
                        nc.vector.tensor_tensor(pv(t2), uim, sn, op=AluOp.mult)
                        nc.vector.tensor_add(vre[:], t1[:], t2[:])
                        t3 = uv_p.tile([128, 2 * C], BF16, tag="uv", name=f"t3_{pj}_{half}")
                        nc.vector.tensor_tensor(pv(t3), uim, cs, op=AluOp.mult)
                        t4 = uv_p.tile([128, 2 * C], BF16, tag="uv", name=f"t4_{pj}_{half}")
                        nc.vector.tensor_tensor(pv(t4), ure, sn, op=AluOp.mult)
                        nc.vector.tensor_sub(vim[:], t3[:], t4[:])

                    # chained scans; slot si's init is gate[si] * (slot
                    # si-1's final state)
                    rho_b = _bcast_cols(rho_sb[:, half:half + 1], C)
                    for plane, vch in ((0, vre), (1, vim)):
                        wp = wcur[plane][half]
                        for sub in range(2):
                            si = 2 * pj + sub
                            scol = slice(sub * C, (sub + 1) * C)
                            if sub == 1:
                                prev_ap = wp[:, C - 1:C]
                            elif pj > 0:
                                prev_ap = wprev[plane][half][:, 2 * C - 1:2 * C]
                            else:
                                prev_ap = None
                            if si == 0:
                                init = 0.0
                            elif si <= static_cont:
                                # schedule guarantees continuation here on
                                # every core: chain directly, no gate
                                init = prev_ap
                            else:
                                g = uv_p.tile([128, 1], F32, tag="g",
                                              name=f"g_{si}_{plane}_{half}")
                                nc.vector.tensor_tensor(
                                    g[:], prev_ap,
                                    gate_sb[:, si:si + 1], op=AluOp.mult)
                                init = g[:, 0:1]
                            nc.vector.tensor_tensor_scan(
                                out=wp[:, scol],
                                data0=rho_b,
                                data1=vch[:, sub * C:(sub + 1) * C],
                                initial=init,
                                op0=AluOp.mult,
                                op1=AluOp.add,
                            )
                            if half == 0 or pj == NP - 1:
                                pulse(wp[:, scol])

                    # s = e^{+i theta l} * w. For all but the last pair the
                    # DVE writes the cos products straight into s and the
                    # sin cross-terms ride SBUF->SBUF accumulate-DMAs (DMA
                    # data path, no DVE port cost). The last pair keeps DVE
                    # adds: its phase-D is the kernel tail and the DMA
                    # round-trip would sit on the critical path.
                    if pj < NP - 1:
                        tcol2 = slice(half * R + 2 * pj * C,
                                      half * R + (2 * pj + 2) * C)
                        nsn = nsin_sb[:, tcol2].rearrange("p (s c) -> p s c", s=2)
                        wre = wcur[0][half][:].rearrange("p (s c) -> p s c", s=2)
                        wim = wcur[1][half][:].rearrange("p (s c) -> p s c", s=2)
                        sre = s_p.tile([128, 2 * C], BF16, tag="sch",
                                       name=f"sre_{pj}_{half}")
                        sim = s_p.tile([128, 2 * C], BF16, tag="sch",
                                       name=f"sim_{pj}_{half}")

                        def pv2(t):
                            return t[:].rearrange("p (s c) -> p s c", s=2)
                        nc.vector.tensor_tensor(pv2(sre), wre, cs, op=AluOp.mult)
                        q2 = uv_p.tile([128, 2 * C], BF16, tag="uv", name=f"q2_{pj}_{half}")
                        nc.vector.tensor_tensor(pv2(q2), wim, nsn, op=AluOp.mult)
                        nc.gpsimd.dma_start(sre[:], q2[:], accum_op=AluOp.add)
                        nc.vector.tensor_tensor(pv2(sim), wim, cs, op=AluOp.mult)
                        q4 = uv_p.tile([128, 2 * C], BF16, tag="uv", name=f"q4_{pj}_{half}")
                        nc.vector.tensor_tensor(pv2(q4), wre, sn, op=AluOp.mult)
                        nc.gpsimd.dma_start(sim[:], q4[:], accum_op=AluOp.add)
                        s_ch[0][half] = sre
                        s_ch[1][half] = sim

                pending = (pj, xt_pair, wcur, s_ch)

            ppj, pxt, wlast, _ = pending
            s_last = emit_back_rot_dve(ppj, wlast)
            emit_phase_d(ppj, s_last, pxt, last=True)
            warm_out = consts.tile([128, 1], F32, tag="warmout")
            nc.vector.tensor_copy(warm_out[:], warm_ps[:, 0:1])

    nc.compile()
    return nc


_NC_CACHE = {}


def _get_nc(key):
    if key not in _NC_CACHE:
        _NC_CACHE[key] = build_nc(*key)
    return _NC_CACHE[key]


# --------------------------------------------------------------------------
# host-side data prep
# --------------------------------------------------------------------------

def _host_prep(lambda_real_log, lambda_imag, log_dt, B_re, B_im, C_re, C_im):
    """Schedule-independent parameter prep: w1, w2, rho, theta."""
    lam_re = -np.exp(np.asarray(lambda_real_log, np.float64))
    lam_im = np.asarray(lambda_imag, np.float64)
    dtv = np.log1p(np.exp(np.float64(log_dt))) + 1e-4
    rho = np.exp(dtv * lam_re)                       # [N]
    theta = dtv * lam_im                             # [N]
    lam = lam_re + 1j * lam_im
    abar = np.exp(dtv * lam)
    bb = ((abar - 1.0) / lam)[:, None] * (
        np.asarray(B_re, np.float64) + 1j * np.asarray(B_im, np.float64)
    )                                                # [N, D] complex
    bb_planes = (np.ascontiguousarray(bb.real), np.ascontiguousarray(bb.imag))

    w1 = np.empty((128, DK * 2 * NH * 128), BF16_NP)
    for k in range(DK):
        for plane in range(2):
            for half in range(NH):
                col = ((k * 2 + plane) * 2 + half) * 128
                w1[:, col:col + 128] = bb_planes[plane][
                    half * 128:(half + 1) * 128, k * 128:(k + 1) * 128
                ].T.astype(np.float32)

    w2 = np.empty((128, 2 * NH * D), BF16_NP)
    c_planes = (np.asarray(C_re, np.float64), -np.asarray(C_im, np.float64))
    for plane in range(2):
        for half in range(NH):
            col = (plane * 2 + half) * D
            w2[:, col:col + D] = c_planes[plane][
                :, half * 128:(half + 1) * 128
            ].T.astype(np.float32)

    rho_in = np.empty((128, NH), np.float32)
    for half in range(NH):
        rho_in[:, half] = rho[half * 128:(half + 1) * 128]

    return w1, w2, rho_in, theta


def _pack_core(slots, x, lengths, theta, M):
    """Per-core packed inputs for one slot list."""
    R = M * C
    RT = R // 128
    xt = np.zeros((D, R), BF16_NP)
    cost = np.empty((128, NH * R), BF16_NP)
    sint = np.empty((128, NH * R), BF16_NP)
    gate = np.zeros((128, M), np.float32)
    maskc = np.zeros((128, RT), np.float32)

    l_idx = np.arange(C, dtype=np.float64)
    for si, s in enumerate(slots):
        cols = slice(si * C, (si + 1) * C)
        if s.kind == "dummy":
            l0 = 0
        else:
            l0 = s.chunk * C
            xs = np.asarray(x[s.batch, l0:l0 + C, :])      # [C, D]
            xt[:, cols] = xs.T.astype(BF16_NP)
            if s.kind == "real":
                ml = np.clip(int(lengths[s.batch]) - l0, 0, C)
                rowmask = (np.arange(C) < ml).astype(np.float32)
                maskc[:, si * 4:(si + 1) * 4] = rowmask.reshape(4, 128).T
        gate[:, si] = s.gate
        for half in range(NH):
            ph = theta[half * 128:(half + 1) * 128, None] * (l0 + l_idx)[None, :]
            tc = slice(half * R + si * C, half * R + (si + 1) * C)
            cost[:, tc] = np.cos(ph).astype(BF16_NP)
            sint[:, tc] = np.sin(ph).astype(BF16_NP)
    return {"xt": xt, "cost": cost, "sint": sint,
            "nsint": np.ascontiguousarray(-sint), "gate": gate, "maskc": maskc}


def prepare(x, lengths, lambda_real_log, lambda_imag, log_dt,
            B_re, B_im, C_re, C_im, D_weight):
    x = np.asarray(x, np.float32)
    Dw = np.asarray(D_weight, np.float32)
    dw_is_eye = bool(
        Dw.shape == (D, D) and np.array_equal(Dw, np.eye(D, dtype=np.float32)))

    dwt = np.empty((128, DK * D), BF16_NP)
    for k in range(DK):
        dwt[:, k * D:(k + 1) * D] = Dw.T[k * 128:(k + 1) * 128, :]

    M, static_cont, cores = plan_schedule(np.asarray(lengths))
    w1, w2, rho_in, theta = _host_prep(
        lambda_real_log, lambda_imag, log_dt, B_re, B_im, C_re, C_im)

    in_maps = []
    for slots in cores:
        m = _pack_core(slots, x, lengths, theta, M)
        m.update({"w1": w1, "w2": w2, "rho": rho_in, "dwt": dwt})
        in_maps.append(m)
    return (M, static_cont, dw_is_eye), cores, in_maps


def unpack_output(res, M, cores):
    y = np.zeros((B, L, D), np.float32)
    for ci, slots in enumerate(cores):
        yc = np.asarray(res.results[ci]["y"], dtype=np.float32)  # [R, D]
        for si, s in enumerate(slots):
            if s.kind == "real":
                l0 = s.chunk * C
                y[s.batch, l0:l0 + C, :] = yc[si * C:(si + 1) * C, :]
    return y


def kernel(x, lengths, lambda_real_log, lambda_imag, log_dt, B_re, B_im,
           C_re, C_im, D_weight):
    key, cores, in_maps = prepare(
        x, lengths, lambda_real_log, lambda_imag, log_dt,
        B_re, B_im, C_re, C_im, D_weight)
    M = key[0]
    nc = _get_nc(key)

    last_err = None
    for attempt in range(4):  # device errors are occasionally transient under axon
        try:
            if not _NC_CACHE.get(("warm",) + key):
                # throwaway execution: first run in a fresh process is
                # regularly ~15% slower (cold device caches / power state)
                run_bass_kernel_spmd(nc, in_maps, core_ids=list(range(NCORES)))
                _NC_CACHE[("warm",) + key] = True
            res = run_bass_kernel_spmd(nc, in_maps, core_ids=list(range(NCORES)))
            break
        except Exception as e:  # noqa: BLE001
            last_err = e
            time.sleep(5 * (attempt + 1))
    else:
        raise last_err
    return unpack_output(res, M, cores)


# revision 18
# speedup vs baseline: 1.1181x; 1.0718x over previous
"""Trainium2 Bass kernel for DiagonalS5SSM.

Math (per batch b; the reference's where(valid,...) is elided — valid is a
prefix mask in l and the output is masked by the same prefix, so the frozen
tail states never reach the output):

    it[l, n]  = sum_d x[b, l, d] * bbar[n, d]          (complex)
    s[l, n]   = abar[n] * s[l-1, n] + it[l, n]         (complex scan)
    y[b, l, :] = mask[l] * (Re(s[l] @ c^T) + x[b, l] @ D^T)

The complex scan is decoupled into two real scans via polar form
abar = rho * e^{i theta} (rho < 1, so no dynamic-range blowup):

    v[l] = e^{-i theta l} * it[l]       (elementwise rotation)
    w[l] = rho * w[l-1] + v[l]          (HW tensor_tensor_scan per plane)
    s[l] = e^{+i theta l} * w[l]        (rotation back)

Chunk-skipping schedule: since y is zero for l >= lengths[b], only
ceil(lengths[b]/512) chunks per batch ever matter. The host packs those
(batch, chunk) units ("slots") onto the 8 cores (chains of one batch stay
on one core, consecutive slots; long chains may split across cores with a
single discarded "warmup" slot whose zero-init error is rho^512 ~ e^-32).
Every core runs the same M-slot program; per-core differences live in the
packed inputs: x^T slices, rotation-table slices, row masks, and a per-slot
"gate" that multiplies the previous slot's final scan state into the next
slot's initial value (gate=1 continues a chain, gate=0 starts fresh).

Sharding: the slot packing is data-parallel over batch with optional
sequence splits; no collectives. Matmuls and elementwise rotations run in
bf16; the scan coefficient rho stays fp32; scan state is fp32 in hardware.
x is pre-transposed on the host so the contraction dim lands on partitions.
Slots are processed in pairs, software-pipelined: phase-D (stage-5 matmul +
mask + x-residual) of the previous pair overlaps the DVE work of the
current pair. y and the x-residual stream in bf16.
"""

import math
import time

import numpy as np
import ml_dtypes

import concourse.bass as bass
import concourse.tile as tile
from concourse import bacc, mybir
from concourse.bass_utils import run_bass_kernel_spmd

B, L, D, N = 16, 2048, 512, 256
NCORES = 8
C = 512                   # chunk (slot) length
NH = N // 128             # n-halves
DK = D // 128             # d-chunks

F32 = mybir.dt.float32
BF16 = mybir.dt.bfloat16
BF16_NP = np.dtype(ml_dtypes.bfloat16)

AluOp = mybir.AluOpType
ACT_COPY = mybir.ActivationFunctionType.Copy


def _bcast_cols(ap: bass.AP, n: int) -> bass.AP:
    """[128, 1] AP -> [128, n] free-broadcast (step-0) AP."""
    return bass.AP(tensor=ap.tensor, offset=ap.offset, ap=[ap.ap[0], [0, n]])


# --------------------------------------------------------------------------
# schedule planning (host)
# --------------------------------------------------------------------------

class Slot:
    """One 512-row unit of work. real: (batch, chunk) whose output is kept.
    warmup: (batch, chunk) processed only to seed the next slot's scan
    state. dummy: padding (all-zero inputs, zero mask, gate 0)."""

    def __init__(self, kind, batch=-1, chunk=-1, gate=0.0):
        self.kind = kind          # "real" | "warmup" | "dummy"
        self.batch = batch
        self.chunk = chunk
        self.gate = gate          # multiply prev slot's final state into init

    def __repr__(self):
        return f"{self.kind[0]}{self.batch}.{self.chunk}g{int(self.gate)}"


def plan_schedule(lengths):
    """Pack per-batch chunk chains onto NCORES cores.

    Returns (M, cores) where cores is a list of NCORES slot-lists, each of
    length M (even).
    """
    lengths = [int(v) for v in lengths]
    nch = [min((l + C - 1) // C, L // C) for l in lengths]
    # chains: (batch, first_real_chunk, n_real, needs_warmup)
    chains = [(b, 0, n, False) for b, n in enumerate(nch) if n > 0]
    total = sum(c[2] for c in chains)
    M = max((total + NCORES - 1) // NCORES, 1)

    def try_pack(M):
        rem = [M] * NCORES
        bins = [[] for _ in range(NCORES)]
        work = sorted(chains, key=lambda c: -(c[2] + c[3]))
        while work:
            ch = work.pop(0)
            b, c0, n, warm = ch
            size = n + (1 if warm else 0)
            # best-fit: smallest remaining capacity that still fits
            cand = [i for i in range(NCORES) if rem[i] >= size]
            if cand:
                i = min(cand, key=lambda i: rem[i])
                bins[i].append(ch)
                rem[i] -= size
                continue
            # must split: use the core with the largest remaining capacity
            i = max(range(NCORES), key=lambda i: rem[i])
            cap = rem[i]
            take_real = cap - (1 if warm else 0)
            if cap < 1 or take_real < 1 or take_real >= n:
                return None
            bins[i].append((b, c0, take_real, warm))
            rem[i] = 0
            # remainder continues on another core behind a warmup slot
            work.insert(0, (b, c0 + take_real, n - take_real, True))
            work.sort(key=lambda c: -(c[2] + c[3]))
        return bins

    while True:
        bins = try_pack(M)
        if bins is not None:
            break
        M += 1
    if M % 2:
        M += 1

    cores = []
    static_cont = M
    for bin_ in bins:
        bin_ = sorted(bin_, key=lambda ch: -(ch[2] + ch[3]))
        if bin_:
            first_slots = bin_[0][2] + (1 if bin_[0][3] else 0)
        else:
            first_slots = 1
        static_cont = min(static_cont, first_slots - 1, 3)
        slots = []
        for (b, c0, n, warm) in bin_:
            if warm:
                slots.append(Slot("warmup", b, c0 - 1, 0.0))
            for j in range(n):
                slots.append(Slot("real", b, c0 + j,
                                  0.0 if (j == 0 and not warm) else 1.0))
        while len(slots) < M:
            slots.append(Slot("dummy"))
        assert len(slots) == M
        cores.append(slots)

    static_cont = max(static_cont, 0)
    for slots in cores:
        for si in range(1, static_cont + 1):
            assert slots[si].gate == 1.0, (static_cont, slots)
    _validate_schedule(lengths, nch, M, cores)
    return M, static_cont, cores


def _validate_schedule(lengths, nch, M, cores):
    seen = {}
    for ci, slots in enumerate(cores):
        state = None  # (batch, last_done_chunk) after each slot
        for s in slots:
            if s.kind == "dummy":
                assert s.gate == 0.0
                state = None
                continue
            if s.gate == 0.0:
                # fresh start: must begin at chunk 0, or be a warmup slot
                assert s.chunk == 0 or s.kind == "warmup", (ci, s)
            else:
                assert state == (s.batch, s.chunk - 1), (ci, s, state)
            state = (s.batch, s.chunk)
            if s.kind == "real":
                assert seen.setdefault((s.batch, s.chunk), ci) == ci
    for b in range(B):
        for c in range(nch[b]):
            assert (b, c) in seen, f"missing chunk {(b, c)}"


# --------------------------------------------------------------------------
# device program (depends only on M)
# --------------------------------------------------------------------------

def build_nc(M, static_cont=0):
    R = M * C                 # rows per core
    RT = R // 128             # 128-row tiles per core
    NP = M // 2               # slot pairs

    nc = bacc.Bacc(
        "TRN2",
        target_bir_lowering=False,
        debug=False,
        enable_asserts=False,
        num_devices=NCORES,
    )

    xt_d = nc.dram_tensor("xt", [D, R], BF16, kind="ExternalInput")
    xadd_d = nc.dram_tensor("xadd", [R, D], BF16, kind="ExternalInput")
    w1_d = nc.dram_tensor("w1", [128, DK * 2 * NH * 128], BF16, kind="ExternalInput")
    w2_d = nc.dram_tensor("w2", [128, 2 * NH * D], BF16, kind="ExternalInput")
    cos_d = nc.dram_tensor("cost", [128, NH * R], BF16, kind="ExternalInput")
    sin_d = nc.dram_tensor("sint", [128, NH * R], BF16, kind="ExternalInput")
    nsin_d = nc.dram_tensor("nsint", [128, NH * R], BF16, kind="ExternalInput")
    rho_d = nc.dram_tensor("rho", [128, NH], F32, kind="ExternalInput")
    gate_d = nc.dram_tensor("gate", [128, M], F32, kind="ExternalInput")
    mask_d = nc.dram_tensor("maskc", [128, RT], F32, kind="ExternalInput")
    y_d = nc.dram_tensor("y", [R, D], BF16, kind="ExternalOutput")

    with tile.TileContext(nc) as tc:
        with (
            tc.tile_pool(name="consts", bufs=1) as consts,
            tc.tile_pool(name="wplanes", bufs=4) as wplanes,
            tc.tile_pool(name="xtp", bufs=3) as xt_p,
            tc.tile_pool(name="uvp", bufs=16) as uv_p,
            tc.tile_pool(name="u2p", bufs=4) as u2_p,
            tc.tile_pool(name="x2p", bufs=2) as x2_p,
            tc.tile_pool(name="sp", bufs=10) as s_p,
            tc.tile_pool(name="yp", bufs=3) as y_p,
            tc.tile_pool(name="ps_it", bufs=2, space="PSUM") as ps_it,
            tc.tile_pool(name="ps_y", bufs=3, space="PSUM") as ps_y,
            tc.tile_pool(name="ps_warm", bufs=1, space="PSUM") as ps_warm,
        ):
            # HAM warmup: the PE clock-gate opens only after ~3.4us of
            # sustained matmul activity. Run throwaway matmuls on a zeroed
            # tile from t~7us (right after the framework preamble) so the
            # real stage-1 stream starts at 2.4 GHz instead of 1.2.
            warm_z = consts.tile([128, 512], BF16, tag="warmz")
            nc.vector.memset(warm_z[:], 0)
            warm_ps = ps_warm.tile([128, 512], F32, tag="warm", name="warmps")
            for wi in range(10):
                nc.tensor.matmul(warm_ps[:], warm_z[:, 0:128], warm_z[:],
                                 start=True, stop=True, skip_group_check=True)

            def pulse(dep_ap):
                # cheap keep-warm matmul paced by a DVE output: keeps the
                # PE HAM window busy during DVE-bound stretches
                nc.tensor.matmul(warm_ps[:, 0:64], warm_z[:, 0:128],
                                 dep_ap[:, 0:64], start=True, stop=True,
                                 skip_group_check=True)
            # Startup-critical loads (w1, first pair's x^T and tables) are
            # spread across all three DMA-capable queues so the first
            # stage-1 matmuls and rotations start as early as possible.
            w1_sb = [consts.tile([128, 2 * NH * 128], BF16, tag=f"w1_{k}",
                                 name=f"w1sb_{k}") for k in range(DK)]

            def w1_load(k, eng):
                eng.dma_start(w1_sb[k][:], w1_d.ap()[:, k * 512:(k + 1) * 512])

            cos_sb = consts.tile([128, NH * R], BF16, tag="cos")
            sin_sb = consts.tile([128, NH * R], BF16, tag="sin")
            nsin_sb = consts.tile([128, NH * R], BF16, tag="nsin")

            def tbl_load(pj, h, eng):
                colsl = slice(h * R + 2 * pj * C, h * R + (2 * pj + 2) * C)
                eng.dma_start(cos_sb[:, colsl], cos_d.ap()[:, colsl])
                eng.dma_start(sin_sb[:, colsl], sin_d.ap()[:, colsl])

            xt_first = []
            for si in range(2):
                t = xt_p.tile([128, DK * C], BF16, tag="xt", name=f"xt_{si}")
                xt_first.append(t)

            # sync queue: first pair's x^T (slot 0 split in two for latency)
            nc.sync.dma_start(
                xt_first[0][:, 0:2 * C].rearrange("p (k c) -> p k c", k=2),
                xt_d.ap()[0:256, 0:C].rearrange("(k p) c -> p k c", p=128))
            nc.sync.dma_start(
                xt_first[0][:, 2 * C:].rearrange("p (k c) -> p k c", k=2),
                xt_d.ap()[256:512, 0:C].rearrange("(k p) c -> p k c", p=128))
            nc.sync.dma_start(
                xt_first[1][:].rearrange("p (k c) -> p k c", k=DK),
                xt_d.ap()[:, C:2 * C].rearrange("(k p) c -> p k c", p=128))
            # scalar queue: w1 chunks then pair-0 tables (half 0)
            w1_load(0, nc.scalar)
            w1_load(1, nc.scalar)
            w1_load(2, nc.scalar)
            tbl_load(0, 0, nc.scalar)
            # gpsimd queue: rho/gate, last w1 chunk, pair-0 tables (half 1),
            # then everything else
            rho_sb = consts.tile([128, NH], F32, tag="rho")
            nc.gpsimd.dma_start(rho_sb[:], rho_d.ap())
            w1_load(3, nc.gpsimd)
            gate_sb = consts.tile([128, M], F32, tag="gate")
            nc.gpsimd.dma_start(gate_sb[:], gate_d.ap())
            tbl_load(0, 1, nc.gpsimd)
            # bulk loads ride the otherwise-idle sync queue so the gpsimd
            # queue stays clear for the runtime accumulate-DMAs that gate
            # phase-D
            w2_sb = consts.tile([128, 2 * NH * D], BF16, tag="w2")
            nc.sync.dma_start(w2_sb[:], w2_d.ap())
            for pj in range(1, NP):
                for h in range(NH):
                    tbl_load(pj, h, nc.sync)
            mask_sb = consts.tile([128, RT], F32, tag="maskc")
            nc.sync.dma_start(mask_sb[:], mask_d.ap())
            for pj in range(NP):
                for h in range(NH):
                    colsl = slice(h * R + 2 * pj * C, h * R + (2 * pj + 2) * C)
                    nc.sync.dma_start(nsin_sb[:, colsl], nsin_d.ap()[:, colsl])

            # full-width scan outputs; slot si owns cols [si*C, (si+1)*C)
            w_pl = [
                [wplanes.tile([128, R], BF16, tag="wpl", name=f"w_{p}_{h}")
                 for h in range(NH)]
                for p in range(2)
            ]

            def emit_phase_d(pj, s_ch, last):
                for sub in range(2):
                    si = 2 * pj + sub
                    rows = slice(si * C, (si + 1) * C)
                    xadd_r = xadd_d.ap()[rows, :].rearrange(
                        "(a p) d -> p a d", p=128)
                    ysb = y_p.tile([128, 4 * D], BF16, tag="ysb", name=f"ysb_{si}")
                    if last:
                        x2 = x2_p.tile([128, 4 * D], BF16, tag="x2", name=f"x2_{si}")
                        nc.sync.dma_start(
                            x2[:].rearrange("p (a d) -> p a d", a=4), xadd_r)
                    for rt2 in range(4):
                        rt = si * 4 + rt2
                        scol = sub * C + rt2 * 128
                        ps = ps_y.tile([128, D], F32, tag="y", name=f"ys_{rt}")
                        first = True
                        for plane in range(2):
                            for half in range(NH):
                                nc.tensor.matmul(
                                    ps[:],
                                    s_ch[plane][half][:, scol:scol + 128],
                                    w2_sb[:, (plane * 2 + half) * D:(plane * 2 + half + 1) * D],
                                    start=first,
                                    stop=(plane == 1 and half == NH - 1),
                                )
                                first = False
                        ycol = slice(rt2 * D, (rt2 + 1) * D)
                        if last:
                            nc.vector.scalar_tensor_tensor(
                                out=ysb[:, ycol], in0=ps[:],
                                scalar=mask_sb[:, rt:rt + 1],
                                in1=x2[:, ycol],
                                op0=AluOp.mult, op1=AluOp.add,
                            )
                            # stream each 128-row tile out as soon as its
                            # masked-add lands: the tail drains incrementally
                            rq = slice(si * C + rt2 * 128, si * C + (rt2 + 1) * 128)
                            nc.scalar.dma_start(
                                y_d.ap()[rq, :], ysb[:, ycol])
                        else:
                            nc.scalar.activation(
                                ysb[:, ycol], ps[:], ACT_COPY,
                                scale=mask_sb[:, rt:rt + 1],
                            )
                    if not last:
                        nc.gpsimd.dma_start(
                            ysb[:].rearrange("p (a d) -> p a d", a=4),
                            xadd_r, accum_op=AluOp.add)
                        nc.scalar.dma_start(
                            y_d.ap()[rows, :].rearrange("(a p) d -> p a d", p=128),
                            ysb[:].rearrange("p (a d) -> p a d", a=4),
                        )


            def _sub2(ap_c, stride_elems):
                # [128, C] AP -> [128, 2, C] view with an outer sub-dim
                return bass.AP(tensor=ap_c.tensor, offset=ap_c.offset,
                               ap=[ap_c.ap[0], [stride_elems, 2], ap_c.ap[1]])

            pending = None
            for pj in range(NP):
                # per-plane u tiles, [sub, half, C] layout: one ACT copy per
                # (slot, plane) moves both halves out of PSUM at once
                u_t = [u2_p.tile([128, 2 * NH * C], BF16, tag="uv2",
                                 name=f"u_{pj}_{plane}")
                       for plane in range(2)]
                for sub in range(2):
                    si = 2 * pj + sub
                    dcol = slice(si * C, (si + 1) * C)
                    if pj == 0:
                        xt = xt_first[sub]
                    else:
                        # one DMA fills all DK d-chunks of this slot's x^T
                        xt = xt_p.tile([128, DK * C], BF16, tag="xt",
                                       name=f"xt_{si}")
                        nc.sync.dma_start(
                            xt[:].rearrange("p (k c) -> p k c", k=DK),
                            xt_d.ap()[:, dcol].rearrange("(k p) c -> p k c", p=128))
                    for plane in range(2):
                        ps = ps_it.tile([128, NH * C], F32, tag="it",
                                        name=f"it_{si}_{plane}")
                        for half in range(NH):
                            for k in range(DK):
                                col = (plane * 2 + half) * 128
                                nc.tensor.matmul(
                                    ps[:, half * C:(half + 1) * C],
                                    w1_sb[k][:, col:col + 128],
                                    xt[:, k * C:(k + 1) * C],
                                    start=(k == 0),
                                    stop=(k == DK - 1),
                                )
                        nc.scalar.activation(
                            u_t[plane][:, sub * NH * C:(sub + 1) * NH * C],
                            ps[:], ACT_COPY)

                # software-pipelined phase-D of the previous pair
                if pending is not None:
                    emit_phase_d(*pending, last=False)

                s_ch = [[None] * NH for _ in range(2)]
                for half in range(NH):
                    tcol = slice(half * R + 2 * pj * C, half * R + (2 * pj + 2) * C)
                    cs = cos_sb[:, tcol].rearrange("p (s c) -> p s c", s=2)
                    sn = sin_sb[:, tcol].rearrange("p (s c) -> p s c", s=2)
                    ure = _sub2(u_t[0][:, half * C:(half + 1) * C], NH * C)
                    uim = _sub2(u_t[1][:, half * C:(half + 1) * C], NH * C)

                    vre = uv_p.tile([128, 2 * C], BF16, tag="uv", name=f"vre_{pj}_{half}")
                    vim = uv_p.tile([128, 2 * C], BF16, tag="uv", name=f"vim_{pj}_{half}")
                    if pj == 0:
                        # first pair: per-slot rotation so the DVE starts as
                        # soon as the FIRST slot's matmuls land
                        for sub in range(2):
                            sl = slice(sub * C, (sub + 1) * C)
                            usl = slice(sub * NH * C + half * C,
                                        sub * NH * C + (half + 1) * C)
                            csl = cos_sb[:, half * R + sub * C: half * R + (sub + 1) * C]
                            snl = sin_sb[:, half * R + sub * C: half * R + (sub + 1) * C]
                            t1 = uv_p.tile([128, C], BF16, tag="uvs",
                                           name=f"t1_{pj}_{half}_{sub}")
                            nc.vector.tensor_tensor(t1[:], u_t[0][:, usl], csl,
                                                    op=AluOp.mult)
                            t2 = uv_p.tile([128, C], BF16, tag="uvs",
                                           name=f"t2_{pj}_{half}_{sub}")
                            nc.vector.tensor_tensor(t2[:], u_t[1][:, usl], snl,
                                                    op=AluOp.mult)
                            t3 = uv_p.tile([128, C], BF16, tag="uvs",
                                           name=f"t3_{pj}_{half}_{sub}")
                            nc.vector.tensor_tensor(t3[:], u_t[1][:, usl], csl,
                                                    op=AluOp.mult)
                            t4 = uv_p.tile([128, C], BF16, tag="uvs",
                                           name=f"t4_{pj}_{half}_{sub}")
                            nc.vector.tensor_tensor(t4[:], u_t[0][:, usl], snl,
                                                    op=AluOp.mult)
                            nc.vector.tensor_add(vre[:, sl], t1[:], t2[:])
                            nc.vector.tensor_sub(vim[:, sl], t3[:], t4[:])
                    else:
                        def pv(t):
                            return t[:].rearrange("p (s c) -> p s c", s=2)
                        t1 = uv_p.tile([128, 2 * C], BF16, tag="uv", name=f"t1_{pj}_{half}")
                        nc.vector.tensor_tensor(pv(t1), ure, cs, op=AluOp.mult)
                        t2 = uv_p.tile([128, 2 * C], BF16, tag="uv", name=f"t2_{pj}_{half}")
                        nc.vector.tensor_tensor(pv(t2), uim, sn, op=AluOp.mult)
                        t3 = uv_p.tile([128, 2 * C], BF16, tag="uv", name=f"t3_{pj}_{half}")
                        nc.vector.tensor_tensor(pv(t3), uim, cs, op=AluOp.mult)
                        t4 = uv_p.tile([128, 2 * C], BF16, tag="uv", name=f"t4_{pj}_{half}")
                        nc.vector.tensor_tensor(pv(t4), ure, sn, op=AluOp.mult)
                        nc.vector.tensor_add(vre[:], t1[:], t2[:])
                        nc.vector.tensor_sub(vim[:], t3[:], t4[:])

                    # chained scans; slot si's init is gate[si] * (slot
                    # si-1's final state)
                    rho_b = _bcast_cols(rho_sb[:, half:half + 1], C)
                    for plane, vch in ((0, vre), (1, vim)):
                        wp = w_pl[plane][half]
                        for sub in range(2):
                            si = 2 * pj + sub
                            scol = slice(si * C, (si + 1) * C)
                            if si == 0:
                                init = 0.0
                            elif si <= static_cont:
                                # schedule guarantees continuation here on
                                # every core: chain directly, no gate
                                init = wp[:, si * C - 1:si * C]
                            else:
                                prev = si * C - 1
                                g = uv_p.tile([128, 1], F32, tag="g",
                                              name=f"g_{si}_{plane}_{half}")
                                nc.vector.tensor_tensor(
                                    g[:], wp[:, prev:prev + 1],
                                    gate_sb[:, si:si + 1], op=AluOp.mult)
                                init = g[:, 0:1]
                            nc.vector.tensor_tensor_scan(
                                out=wp[:, scol],
                                data0=rho_b,
                                data1=vch[:, sub * C:(sub + 1) * C],
                                initial=init,
                                op0=AluOp.mult,
                                op1=AluOp.add,
                            )
                            # pulse only on the first half (and the final
                            # pair): later pulses would hold back the next
                            # pair's stage-1 matmuls in the in-order PE
                            # queue; those real matmuls keep the PE warm
                            # through the second half instead
                            if half == 0 or pj == NP - 1:
                                pulse(wp[:, scol])

                    # s = e^{+i theta l} * w. For all but the last pair the
                    # DVE writes the cos products straight into s and the
                    # sin cross-terms ride SBUF->SBUF accumulate-DMAs (DMA
                    # data path, no DVE port cost). The last pair keeps DVE
                    # adds: its phase-D is the kernel tail and the DMA
                    # round-trip would sit on the critical path.
                    pcol = slice(2 * pj * C, (2 * pj + 2) * C)
                    wre = w_pl[0][half][:, pcol].rearrange("p (s c) -> p s c", s=2)
                    wim = w_pl[1][half][:, pcol].rearrange("p (s c) -> p s c", s=2)
                    nsn = nsin_sb[:, tcol].rearrange("p (s c) -> p s c", s=2)
                    sre = s_p.tile([128, 2 * C], BF16, tag="sch",
                                   name=f"sre_{pj}_{half}")
                    sim = s_p.tile([128, 2 * C], BF16, tag="sch",
                                   name=f"sim_{pj}_{half}")
                    if pj < NP - 1:
                        def pv2(t):
                            return t[:].rearrange("p (s c) -> p s c", s=2)
                        sre_v = pv2(sre)
                        sim_v = pv2(sim)
                        nc.vector.tensor_tensor(sre_v, wre, cs, op=AluOp.mult)
                        q2 = uv_p.tile([128, 2 * C], BF16, tag="uv", name=f"q2_{pj}_{half}")
                        nc.vector.tensor_tensor(pv2(q2), wim, nsn, op=AluOp.mult)
                        nc.gpsimd.dma_start(sre[:], q2[:], accum_op=AluOp.add)
                        nc.vector.tensor_tensor(sim_v, wim, cs, op=AluOp.mult)
                        q4 = uv_p.tile([128, 2 * C], BF16, tag="uv", name=f"q4_{pj}_{half}")
                        nc.vector.tensor_tensor(pv2(q4), wre, sn, op=AluOp.mult)
                        nc.gpsimd.dma_start(sim[:], q4[:], accum_op=AluOp.add)
                    else:
                        # last pair: per-slot DVE ops so slot 2pj's phase-D
                        # matmuls start while slot 2pj+1 is still rotating
                        for sub in range(2):
                            sl = slice(sub * C, (sub + 1) * C)
                            si = 2 * pj + sub
                            css = cos_sb[:, half * R + si * C: half * R + (si + 1) * C]
                            sns = sin_sb[:, half * R + si * C: half * R + (si + 1) * C]
                            wres = w_pl[0][half][:, si * C:(si + 1) * C]
                            wims = w_pl[1][half][:, si * C:(si + 1) * C]
                            q1 = uv_p.tile([128, C], BF16, tag="uvs", name=f"q1_{pj}_{half}_{sub}")
                            nc.vector.tensor_tensor(q1[:], wres, css, op=AluOp.mult)
                            q2 = uv_p.tile([128, C], BF16, tag="uvs", name=f"q2_{pj}_{half}_{sub}")
                            nc.vector.tensor_tensor(q2[:], wims, sns, op=AluOp.mult)
                            nc.vector.tensor_sub(sre[:, sl], q1[:], q2[:])
                            q3 = uv_p.tile([128, C], BF16, tag="uvs", name=f"q3_{pj}_{half}_{sub}")
                            nc.vector.tensor_tensor(q3[:], wims, css, op=AluOp.mult)
                            q4 = uv_p.tile([128, C], BF16, tag="uvs", name=f"q4_{pj}_{half}_{sub}")
                            nc.vector.tensor_tensor(q4[:], wres, sns, op=AluOp.mult)
                            nc.vector.tensor_add(sim[:, sl], q3[:], q4[:])
                    s_ch[0][half] = sre
                    s_ch[1][half] = sim

                pending = (pj, s_ch)

            emit_phase_d(*pending, last=True)
            warm_out = consts.tile([128, 1], F32, tag="warmout")
            nc.vector.tensor_copy(warm_out[:], warm_ps[:, 0:1])

    nc.compile()
    return nc


_NC_CACHE = {}


def _get_nc(key):
    if key not in _NC_CACHE:
        _NC_CACHE[key] = build_nc(*key)
    return _NC_CACHE[key]


# --------------------------------------------------------------------------
# host-side data prep
# --------------------------------------------------------------------------

def _host_prep(lambda_real_log, lambda_imag, log_dt, B_re, B_im, C_re, C_im):
    """Schedule-independent parameter prep: w1, w2, rho, theta."""
    lam_re = -np.exp(np.asarray(lambda_real_log, np.float64))
    lam_im = np.asarray(lambda_imag, np.float64)
    dtv = np.log1p(np.exp(np.float64(log_dt))) + 1e-4
    rho = np.exp(dtv * lam_re)                       # [N]
    theta = dtv * lam_im                             # [N]
    lam = lam_re + 1j * lam_im
    abar = np.exp(dtv * lam)
    bb = ((abar - 1.0) / lam)[:, None] * (
        np.asarray(B_re, np.float64) + 1j * np.asarray(B_im, np.float64)
    )                                                # [N, D] complex
    bb_planes = (np.ascontiguousarray(bb.real), np.ascontiguousarray(bb.imag))

    w1 = np.empty((128, DK * 2 * NH * 128), BF16_NP)
    for k in range(DK):
        for plane in range(2):
            for half in range(NH):
                col = ((k * 2 + plane) * 2 + half) * 128
                w1[:, col:col + 128] = bb_planes[plane][
                    half * 128:(half + 1) * 128, k * 128:(k + 1) * 128
                ].T.astype(np.float32)

    w2 = np.empty((128, 2 * NH * D), BF16_NP)
    c_planes = (np.asarray(C_re, np.float64), -np.asarray(C_im, np.float64))
    for plane in range(2):
        for half in range(NH):
            col = (plane * 2 + half) * D
            w2[:, col:col + D] = c_planes[plane][
                :, half * 128:(half + 1) * 128
            ].T.astype(np.float32)

    rho_in = np.empty((128, NH), np.float32)
    for half in range(NH):
        rho_in[:, half] = rho[half * 128:(half + 1) * 128]

    return w1, w2, rho_in, theta


def _pack_core(slots, x, lengths, theta, M):
    """Per-core packed inputs for one slot list."""
    R = M * C
    RT = R // 128
    xt = np.zeros((D, R), BF16_NP)
    xadd = np.zeros((R, D), BF16_NP)
    cost = np.empty((128, NH * R), BF16_NP)
    sint = np.empty((128, NH * R), BF16_NP)
    gate = np.zeros((128, M), np.float32)
    maskc = np.zeros((128, RT), np.float32)

    l_idx = np.arange(C, dtype=np.float64)
    for si, s in enumerate(slots):
        cols = slice(si * C, (si + 1) * C)
        if s.kind == "dummy":
            l0 = 0
        else:
            l0 = s.chunk * C
            xs = np.asarray(x[s.batch, l0:l0 + C, :])      # [C, D]
            xt[:, cols] = xs.T.astype(BF16_NP)
            if s.kind == "real":
                ml = np.clip(int(lengths[s.batch]) - l0, 0, C)
                rowmask = (np.arange(C) < ml).astype(np.float32)
                maskc[:, si * 4:(si + 1) * 4] = rowmask.reshape(4, 128).T
                xadd[si * C:(si + 1) * C, :] = (
                    xs * rowmask[:, None]).astype(BF16_NP)
        gate[:, si] = s.gate
        for half in range(NH):
            ph = theta[half * 128:(half + 1) * 128, None] * (l0 + l_idx)[None, :]
            tc = slice(half * R + si * C, half * R + (si + 1) * C)
            cost[:, tc] = np.cos(ph).astype(BF16_NP)
            sint[:, tc] = np.sin(ph).astype(BF16_NP)
    return {"xt": xt, "xadd": xadd, "cost": cost, "sint": sint,
            "nsint": np.ascontiguousarray(-sint), "gate": gate, "maskc": maskc}


def prepare(x, lengths, lambda_real_log, lambda_imag, log_dt,
            B_re, B_im, C_re, C_im, D_weight):
    x = np.asarray(x, np.float32)
    Dw = np.asarray(D_weight, np.float32)
    if not (Dw.shape == (D, D) and np.array_equal(Dw, np.eye(D, dtype=np.float32))):
        x_res = (x.reshape(B * L, D) @ Dw.T).reshape(B, L, D)
    else:
        x_res = x

    M, static_cont, cores = plan_schedule(np.asarray(lengths))
    w1, w2, rho_in, theta = _host_prep(
        lambda_real_log, lambda_imag, log_dt, B_re, B_im, C_re, C_im)

    in_maps = []
    for slots in cores:
        m = _pack_core(slots, x, lengths, theta, M)
        # xadd carries the masked D-term (x @ D^T); x itself feeds the SSM
        if x_res is not x:
            R = M * C
            xadd = np.zeros((R, D), BF16_NP)
            for si, s in enumerate(slots):
                if s.kind == "real":
                    l0 = s.chunk * C
                    ml = np.clip(int(lengths[s.batch]) - l0, 0, C)
                    rowmask = (np.arange(C) < ml).astype(np.float32)
                    xadd[si * C:(si + 1) * C, :] = (
                        np.asarray(x_res[s.batch, l0:l0 + C, :])
                        * rowmask[:, None]).astype(BF16_NP)
            m["xadd"] = xadd
        m.update({"w1": w1, "w2": w2, "rho": rho_in})
        in_maps.append(m)
    return (M, static_cont), cores, in_maps


def unpack_output(res, M, cores):
    y = np.zeros((B, L, D), np.float32)
    for ci, slots in enumerate(cores):
        yc = np.asarray(res.results[ci]["y"], dtype=np.float32)  # [R, D]
        for si, s in enumerate(slots):
            if s.kind == "real":
                l0 = s.chunk * C
                y[s.batch, l0:l0 + C, :] = yc[si * C:(si + 1) * C, :]
    return y


def kernel(x, lengths, lambda_real_log, lambda_imag, log_dt, B_re, B_im,
           C_re, C_im, D_weight):
    key, cores, in_maps = prepare(
        x, lengths, lambda_real_log, lambda_imag, log_dt,
        B_re, B_im, C_re, C_im, D_weight)
    M = key[0]
    nc = _get_nc(key)

    last_err = None
    for attempt in range(4):  # device errors are occasionally transient under axon
        try:
            if not _NC_CACHE.get(("warm",) + key):
                # throwaway execution: first run in a fresh process is
                # regularly ~15% slower (cold device caches / power state)
                run_bass_kernel_spmd(nc, in_maps, core_ids=list(range(NCORES)))
                _NC_CACHE[("warm",) + key] = True
            res = run_bass_kernel_spmd(nc, in_maps, core_ids=list(range(NCORES)))
            break
        except Exception as e:  # noqa: BLE001
            last_err = e
            time.sleep(5 * (attempt + 1))
    else:
        raise last_err
    return unpack_output(res, M, cores)



# revision 19
# speedup vs baseline: 1.2711x; 1.1368x over previous
"""Trainium2 Bass kernel for DiagonalS5SSM.

Math (per batch b; the reference's where(valid,...) is elided — valid is a
prefix mask in l and the output is masked by the same prefix, so the frozen
tail states never reach the output):

    it[l, n]  = sum_d x[b, l, d] * bbar[n, d]          (complex)
    s[l, n]   = abar[n] * s[l-1, n] + it[l, n]         (complex scan)
    y[b, l, :] = mask[l] * (Re(s[l] @ c^T) + x[b, l] @ D^T)

The complex scan is decoupled into two real scans via polar form
abar = rho * e^{i theta} (rho < 1, so no dynamic-range blowup):

    v[l] = e^{-i theta l} * it[l]       (elementwise rotation)
    w[l] = rho * w[l-1] + v[l]          (HW tensor_tensor_scan per plane)
    s[l] = e^{+i theta l} * w[l]        (rotation back)

Chunk-skipping schedule: since y is zero for l >= lengths[b], only
ceil(lengths[b]/512) chunks per batch ever matter. The host packs those
(batch, chunk) units ("slots") onto the 8 cores (chains of one batch stay
on one core, consecutive slots; long chains may split across cores with a
single discarded "warmup" slot whose zero-init error is rho^512 ~ e^-32).
Every core runs the same M-slot program; per-core differences live in the
packed inputs: x^T slices, rotation-table slices, row masks, and a per-slot
"gate" that multiplies the previous slot's final scan state into the next
slot's initial value (gate=1 continues a chain, gate=0 starts fresh).

Sharding: the slot packing is data-parallel over batch with optional
sequence splits; no collectives. Matmuls and elementwise rotations run in
bf16; the scan coefficient rho stays fp32; scan state is fp32 in hardware.
x is pre-transposed on the host so the contraction dim lands on partitions.
Slots are processed in pairs, software-pipelined: phase-D (stage-5 matmul +
mask + x-residual) of the previous pair overlaps the DVE work of the
current pair. y and the x-residual stream in bf16.
"""

import math
import time

import numpy as np
import ml_dtypes

import concourse.bass as bass
import concourse.tile as tile
from concourse import bacc, mybir
from concourse.bass_utils import run_bass_kernel_spmd

B, L, D, N = 16, 2048, 512, 256
NCORES = 8
C = 512                   # chunk (slot) length
NH = N // 128             # n-halves
DK = D // 128             # d-chunks

F32 = mybir.dt.float32
BF16 = mybir.dt.bfloat16
BF16_NP = np.dtype(ml_dtypes.bfloat16)

AluOp = mybir.AluOpType
ACT_COPY = mybir.ActivationFunctionType.Copy


def _bcast_cols(ap: bass.AP, n: int) -> bass.AP:
    """[128, 1] AP -> [128, n] free-broadcast (step-0) AP."""
    return bass.AP(tensor=ap.tensor, offset=ap.offset, ap=[ap.ap[0], [0, n]])


# --------------------------------------------------------------------------
# schedule planning (host)
# --------------------------------------------------------------------------

class Slot:
    """One 512-row unit of work. real: (batch, chunk) whose output is kept.
    warmup: (batch, chunk) processed only to seed the next slot's scan
    state. dummy: padding (all-zero inputs, zero mask, gate 0)."""

    def __init__(self, kind, batch=-1, chunk=-1, gate=0.0):
        self.kind = kind          # "real" | "warmup" | "dummy"
        self.batch = batch
        self.chunk = chunk
        self.gate = gate          # multiply prev slot's final state into init

    def __repr__(self):
        return f"{self.kind[0]}{self.batch}.{self.chunk}g{int(self.gate)}"


def plan_schedule(lengths):
    """Pack per-batch chunk chains onto NCORES cores.

    Returns (M, cores) where cores is a list of NCORES slot-lists, each of
    length M (even).
    """
    lengths = [int(v) for v in lengths]
    nch = [min((l + C - 1) // C, L // C) for l in lengths]
    # chains: (batch, first_real_chunk, n_real, needs_warmup)
    chains = [(b, 0, n, False) for b, n in enumerate(nch) if n > 0]
    total = sum(c[2] for c in chains)
    M = max((total + NCORES - 1) // NCORES, 1)

    def try_pack(M):
        rem = [M] * NCORES
        bins = [[] for _ in range(NCORES)]
        work = sorted(chains, key=lambda c: -(c[2] + c[3]))
        while work:
            ch = work.pop(0)
            b, c0, n, warm = ch
            size = n + (1 if warm else 0)
            # best-fit: smallest remaining capacity that still fits
            cand = [i for i in range(NCORES) if rem[i] >= size]
            if cand:
                i = min(cand, key=lambda i: rem[i])
                bins[i].append(ch)
                rem[i] -= size
                continue
            # must split: use the core with the largest remaining capacity
            i = max(range(NCORES), key=lambda i: rem[i])
            cap = rem[i]
            take_real = cap - (1 if warm else 0)
            if cap < 1 or take_real < 1 or take_real >= n:
                return None
            bins[i].append((b, c0, take_real, warm))
            rem[i] = 0
            # remainder continues on another core behind a warmup slot
            work.insert(0, (b, c0 + take_real, n - take_real, True))
            work.sort(key=lambda c: -(c[2] + c[3]))
        return bins

    while True:
        bins = try_pack(M)
        if bins is not None:
            break
        M += 1
    if M % 2:
        M += 1

    cores = []
    static_cont = M
    for bin_ in bins:
        bin_ = sorted(bin_, key=lambda ch: -(ch[2] + ch[3]))
        if bin_:
            first_slots = bin_[0][2] + (1 if bin_[0][3] else 0)
        else:
            first_slots = 1
        static_cont = min(static_cont, first_slots - 1, 3)
        slots = []
        for (b, c0, n, warm) in bin_:
            if warm:
                slots.append(Slot("warmup", b, c0 - 1, 0.0))
            for j in range(n):
                slots.append(Slot("real", b, c0 + j,
                                  0.0 if (j == 0 and not warm) else 1.0))
        while len(slots) < M:
            slots.append(Slot("dummy"))
        assert len(slots) == M
        cores.append(slots)

    static_cont = max(static_cont, 0)
    for slots in cores:
        for si in range(1, static_cont + 1):
            assert slots[si].gate == 1.0, (static_cont, slots)
    _validate_schedule(lengths, nch, M, cores)
    return M, static_cont, cores


def _validate_schedule(lengths, nch, M, cores):
    seen = {}
    for ci, slots in enumerate(cores):
        state = None  # (batch, last_done_chunk) after each slot
        for s in slots:
            if s.kind == "dummy":
                assert s.gate == 0.0
                state = None
                continue
            if s.gate == 0.0:
                # fresh start: must begin at chunk 0, or be a warmup slot
                assert s.chunk == 0 or s.kind == "warmup", (ci, s)
            else:
                assert state == (s.batch, s.chunk - 1), (ci, s, state)
            state = (s.batch, s.chunk)
            if s.kind == "real":
                assert seen.setdefault((s.batch, s.chunk), ci) == ci
    for b in range(B):
        for c in range(nch[b]):
            assert (b, c) in seen, f"missing chunk {(b, c)}"


# --------------------------------------------------------------------------
# device program (depends only on M)
# --------------------------------------------------------------------------

def build_nc(M, static_cont=0):
    R = M * C                 # rows per core
    RT = R // 128             # 128-row tiles per core
    NP = M // 2               # slot pairs

    nc = bacc.Bacc(
        "TRN2",
        target_bir_lowering=False,
        debug=False,
        enable_asserts=False,
        num_devices=NCORES,
    )

    xt_d = nc.dram_tensor("xt", [D, R], BF16, kind="ExternalInput")
    xadd_d = nc.dram_tensor("xadd", [R, D], BF16, kind="ExternalInput")
    w1_d = nc.dram_tensor("w1", [128, DK * 2 * NH * 128], BF16, kind="ExternalInput")
    w2_d = nc.dram_tensor("w2", [128, 2 * NH * D], BF16, kind="ExternalInput")
    cos_d = nc.dram_tensor("cost", [128, NH * R], BF16, kind="ExternalInput")
    sin_d = nc.dram_tensor("sint", [128, NH * R], BF16, kind="ExternalInput")
    nsin_d = nc.dram_tensor("nsint", [128, NH * R], BF16, kind="ExternalInput")
    rho_d = nc.dram_tensor("rho", [128, NH], F32, kind="ExternalInput")
    gate_d = nc.dram_tensor("gate", [128, M], F32, kind="ExternalInput")
    mask_d = nc.dram_tensor("maskc", [128, RT], F32, kind="ExternalInput")
    y_d = nc.dram_tensor("y", [R, D], BF16, kind="ExternalOutput")

    with tile.TileContext(nc) as tc:
        with (
            tc.tile_pool(name="consts", bufs=1) as consts,
            tc.tile_pool(name="wplanes", bufs=4) as wplanes,
            tc.tile_pool(name="xtp", bufs=3) as xt_p,
            tc.tile_pool(name="uvp", bufs=16) as uv_p,
            tc.tile_pool(name="u2p", bufs=4) as u2_p,
            tc.tile_pool(name="x2p", bufs=2) as x2_p,
            tc.tile_pool(name="sp", bufs=10) as s_p,
            tc.tile_pool(name="yp", bufs=3) as y_p,
            tc.tile_pool(name="ps_it", bufs=2, space="PSUM") as ps_it,
            tc.tile_pool(name="ps_y", bufs=3, space="PSUM") as ps_y,
            tc.tile_pool(name="ps_warm", bufs=1, space="PSUM") as ps_warm,
        ):
            # HAM warmup: the PE clock-gate opens only after ~3.4us of
            # sustained matmul activity. Run throwaway matmuls on a zeroed
            # tile from t~7us (right after the framework preamble) so the
            # real stage-1 stream starts at 2.4 GHz instead of 1.2.
            warm_z = consts.tile([128, 512], BF16, tag="warmz")
            nc.vector.memset(warm_z[:], 0)
            warm_ps = ps_warm.tile([128, 512], F32, tag="warm", name="warmps")
            for wi in range(10):
                nc.tensor.matmul(warm_ps[:], warm_z[:, 0:128], warm_z[:],
                                 start=True, stop=True, skip_group_check=True)

            def pulse(dep_ap):
                # cheap keep-warm matmul paced by a DVE output: keeps the
                # PE HAM window busy during DVE-bound stretches
                nc.tensor.matmul(warm_ps[:, 0:64], warm_z[:, 0:128],
                                 dep_ap[:, 0:64], start=True, stop=True,
                                 skip_group_check=True)
            # Startup-critical loads (w1, first pair's x^T and tables) are
            # spread across all three DMA-capable queues so the first
            # stage-1 matmuls and rotations start as early as possible.
            w1_sb = [consts.tile([128, 2 * NH * 128], BF16, tag=f"w1_{k}",
                                 name=f"w1sb_{k}") for k in range(DK)]

            def w1_load(k, eng):
                eng.dma_start(w1_sb[k][:], w1_d.ap()[:, k * 512:(k + 1) * 512])

            cos_sb = consts.tile([128, NH * R], BF16, tag="cos")
            sin_sb = consts.tile([128, NH * R], BF16, tag="sin")
            nsin_sb = consts.tile([128, NH * R], BF16, tag="nsin")

            def tbl_load(pj, h, eng):
                colsl = slice(h * R + 2 * pj * C, h * R + (2 * pj + 2) * C)
                eng.dma_start(cos_sb[:, colsl], cos_d.ap()[:, colsl])
                eng.dma_start(sin_sb[:, colsl], sin_d.ap()[:, colsl])

            xt_first = []
            for si in range(2):
                t = xt_p.tile([128, DK * C], BF16, tag="xt", name=f"xt_{si}")
                xt_first.append(t)

            # sync queue: first pair's x^T (slot 0 split in two for latency)
            nc.sync.dma_start(
                xt_first[0][:, 0:2 * C].rearrange("p (k c) -> p k c", k=2),
                xt_d.ap()[0:256, 0:C].rearrange("(k p) c -> p k c", p=128))
            nc.sync.dma_start(
                xt_first[0][:, 2 * C:].rearrange("p (k c) -> p k c", k=2),
                xt_d.ap()[256:512, 0:C].rearrange("(k p) c -> p k c", p=128))
            nc.sync.dma_start(
                xt_first[1][:].rearrange("p (k c) -> p k c", k=DK),
                xt_d.ap()[:, C:2 * C].rearrange("(k p) c -> p k c", p=128))
            # scalar queue: w1 chunks then pair-0 tables (half 0)
            w1_load(0, nc.scalar)
            w1_load(1, nc.scalar)
            w1_load(2, nc.scalar)
            tbl_load(0, 0, nc.scalar)
            # gpsimd queue: rho/gate, last w1 chunk, pair-0 tables (half 1),
            # then everything else
            rho_sb = consts.tile([128, NH], F32, tag="rho")
            nc.gpsimd.dma_start(rho_sb[:], rho_d.ap())
            w1_load(3, nc.gpsimd)
            gate_sb = consts.tile([128, M], F32, tag="gate")
            nc.gpsimd.dma_start(gate_sb[:], gate_d.ap())
            tbl_load(0, 1, nc.gpsimd)
            # bulk loads ride the otherwise-idle sync queue so the gpsimd
            # queue stays clear for the runtime accumulate-DMAs that gate
            # phase-D
            w2_sb = consts.tile([128, 2 * NH * D], BF16, tag="w2")
            nc.sync.dma_start(w2_sb[:], w2_d.ap())
            for pj in range(1, NP):
                for h in range(NH):
                    tbl_load(pj, h, nc.sync)
            mask_sb = consts.tile([128, RT], F32, tag="maskc")
            nc.sync.dma_start(mask_sb[:], mask_d.ap())
            for pj in range(NP):
                for h in range(NH):
                    colsl = slice(h * R + 2 * pj * C, h * R + (2 * pj + 2) * C)
                    nc.sync.dma_start(nsin_sb[:, colsl], nsin_d.ap()[:, colsl])

            # full-width scan outputs; slot si owns cols [si*C, (si+1)*C)
            w_pl = [
                [wplanes.tile([128, R], BF16, tag="wpl", name=f"w_{p}_{h}")
                 for h in range(NH)]
                for p in range(2)
            ]

            def emit_phase_d(pj, s_ch, last):
                for sub in range(2):
                    si = 2 * pj + sub
                    rows = slice(si * C, (si + 1) * C)
                    xadd_r = xadd_d.ap()[rows, :].rearrange(
                        "(a p) d -> p a d", p=128)
                    ysb = y_p.tile([128, 4 * D], BF16, tag="ysb", name=f"ysb_{si}")
                    if last:
                        x2 = x2_p.tile([128, 4 * D], BF16, tag="x2", name=f"x2_{si}")
                        nc.sync.dma_start(
                            x2[:].rearrange("p (a d) -> p a d", a=4), xadd_r)
                    for rt2 in range(4):
                        rt = si * 4 + rt2
                        scol = sub * C + rt2 * 128
                        ps = ps_y.tile([128, D], F32, tag="y", name=f"ys_{rt}")
                        first = True
                        for plane in range(2):
                            for half in range(NH):
                                nc.tensor.matmul(
                                    ps[:],
                                    s_ch[plane][half][:, scol:scol + 128],
                                    w2_sb[:, (plane * 2 + half) * D:(plane * 2 + half + 1) * D],
                                    start=first,
                                    stop=(plane == 1 and half == NH - 1),
                                )
                                first = False
                        ycol = slice(rt2 * D, (rt2 + 1) * D)
                        if last:
                            nc.vector.scalar_tensor_tensor(
                                out=ysb[:, ycol], in0=ps[:],
                                scalar=mask_sb[:, rt:rt + 1],
                                in1=x2[:, ycol],
                                op0=AluOp.mult, op1=AluOp.add,
                            )
                            # stream each 128-row tile out as soon as its
                            # masked-add lands: the tail drains incrementally
                            rq = slice(si * C + rt2 * 128, si * C + (rt2 + 1) * 128)
                            nc.scalar.dma_start(
                                y_d.ap()[rq, :], ysb[:, ycol])
                        else:
                            nc.scalar.activation(
                                ysb[:, ycol], ps[:], ACT_COPY,
                                scale=mask_sb[:, rt:rt + 1],
                            )
                    if not last:
                        nc.gpsimd.dma_start(
                            ysb[:].rearrange("p (a d) -> p a d", a=4),
                            xadd_r, accum_op=AluOp.add)
                        nc.scalar.dma_start(
                            y_d.ap()[rows, :].rearrange("(a p) d -> p a d", p=128),
                            ysb[:].rearrange("p (a d) -> p a d", a=4),
                        )


            def _sub2(ap_c, stride_elems):
                # [128, C] AP -> [128, 2, C] view with an outer sub-dim
                return bass.AP(tensor=ap_c.tensor, offset=ap_c.offset,
                               ap=[ap_c.ap[0], [stride_elems, 2], ap_c.ap[1]])

            pending = None
            for pj in range(NP):
                # per-plane u tiles, [sub, half, C] layout: one ACT copy per
                # (slot, plane) moves both halves out of PSUM at once
                u_t = [u2_p.tile([128, 2 * NH * C], BF16, tag="uv2",
                                 name=f"u_{pj}_{plane}")
                       for plane in range(2)]
                for sub in range(2):
                    si = 2 * pj + sub
                    dcol = slice(si * C, (si + 1) * C)
                    if pj == 0:
                        xt = xt_first[sub]
                    else:
                        # one DMA fills all DK d-chunks of this slot's x^T
                        xt = xt_p.tile([128, DK * C], BF16, tag="xt",
                                       name=f"xt_{si}")
                        nc.sync.dma_start(
                            xt[:].rearrange("p (k c) -> p k c", k=DK),
                            xt_d.ap()[:, dcol].rearrange("(k p) c -> p k c", p=128))
                    for plane in range(2):
                        ps = ps_it.tile([128, NH * C], F32, tag="it",
                                        name=f"it_{si}_{plane}")
                        for half in range(NH):
                            for k in range(DK):
                                col = (plane * 2 + half) * 128
                                nc.tensor.matmul(
                                    ps[:, half * C:(half + 1) * C],
                                    w1_sb[k][:, col:col + 128],
                                    xt[:, k * C:(k + 1) * C],
                                    start=(k == 0),
                                    stop=(k == DK - 1),
                                )
                        nc.scalar.activation(
                            u_t[plane][:, sub * NH * C:(sub + 1) * NH * C],
                            ps[:], ACT_COPY)

                # software-pipelined phase-D of the previous pair
                if pending is not None:
                    emit_phase_d(*pending, last=False)

                s_ch = [[None] * NH for _ in range(2)]
                v_pl = [[None] * NH for _ in range(2)]
                for half in range(NH):
                    tcol = slice(half * R + 2 * pj * C, half * R + (2 * pj + 2) * C)
                    cs = cos_sb[:, tcol].rearrange("p (s c) -> p s c", s=2)
                    sn = sin_sb[:, tcol].rearrange("p (s c) -> p s c", s=2)
                    nsn = nsin_sb[:, tcol].rearrange("p (s c) -> p s c", s=2)
                    ure = _sub2(u_t[0][:, half * C:(half + 1) * C], NH * C)
                    uim = _sub2(u_t[1][:, half * C:(half + 1) * C], NH * C)

                    vre = uv_p.tile([128, 2 * C], BF16, tag="uv", name=f"vre_{pj}_{half}")
                    vim = uv_p.tile([128, 2 * C], BF16, tag="uv", name=f"vim_{pj}_{half}")
                    if pj == 0:
                        # first pair: per-slot rotation so the DVE starts as
                        # soon as the FIRST slot's matmuls land
                        for sub in range(2):
                            sl = slice(sub * C, (sub + 1) * C)
                            usl = slice(sub * NH * C + half * C,
                                        sub * NH * C + (half + 1) * C)
                            csl = cos_sb[:, half * R + sub * C: half * R + (sub + 1) * C]
                            snl = sin_sb[:, half * R + sub * C: half * R + (sub + 1) * C]
                            t1 = uv_p.tile([128, C], BF16, tag="uvs",
                                           name=f"t1_{pj}_{half}_{sub}")
                            nc.vector.tensor_tensor(t1[:], u_t[0][:, usl], csl,
                                                    op=AluOp.mult)
                            t2 = uv_p.tile([128, C], BF16, tag="uvs",
                                           name=f"t2_{pj}_{half}_{sub}")
                            nc.vector.tensor_tensor(t2[:], u_t[1][:, usl], snl,
                                                    op=AluOp.mult)
                            t3 = uv_p.tile([128, C], BF16, tag="uvs",
                                           name=f"t3_{pj}_{half}_{sub}")
                            nc.vector.tensor_tensor(t3[:], u_t[1][:, usl], csl,
                                                    op=AluOp.mult)
                            t4 = uv_p.tile([128, C], BF16, tag="uvs",
                                           name=f"t4_{pj}_{half}_{sub}")
                            nc.vector.tensor_tensor(t4[:], u_t[0][:, usl], snl,
                                                    op=AluOp.mult)
                            nc.vector.tensor_add(vre[:, sl], t1[:], t2[:])
                            nc.vector.tensor_sub(vim[:, sl], t3[:], t4[:])
                    else:
                        # cos products straight into v; sin cross-terms ride
                        # SBUF->SBUF accumulate-DMAs (same trick as the back
                        # rotation) — saves two DVE adds per half
                        def pv(t):
                            return t[:].rearrange("p (s c) -> p s c", s=2)
                        nc.vector.tensor_tensor(pv(vre), ure, cs, op=AluOp.mult)
                        t2 = uv_p.tile([128, 2 * C], BF16, tag="uv", name=f"t2_{pj}_{half}")
                        nc.vector.tensor_tensor(pv(t2), uim, sn, op=AluOp.mult)
                        nc.gpsimd.dma_start(vre[:], t2[:], accum_op=AluOp.add)
                        nc.vector.tensor_tensor(pv(vim), uim, cs, op=AluOp.mult)
                        t4 = uv_p.tile([128, 2 * C], BF16, tag="uv", name=f"t4_{pj}_{half}")
                        nc.vector.tensor_tensor(pv(t4), ure, nsn, op=AluOp.mult)
                        nc.gpsimd.dma_start(vim[:], t4[:], accum_op=AluOp.add)
                    v_pl[0][half] = vre
                    v_pl[1][half] = vim

                for half in range(NH):
                    # chained scans; slot si's init is gate[si] * (slot
                    # si-1's final state)
                    rho_b = _bcast_cols(rho_sb[:, half:half + 1], C)
                    for plane in range(2):
                        vch = v_pl[plane][half]
                        wp = w_pl[plane][half]
                        for sub in range(2):
                            si = 2 * pj + sub
                            scol = slice(si * C, (si + 1) * C)
                            if si == 0:
                                init = 0.0
                            elif si <= static_cont:
                                # schedule guarantees continuation here on
                                # every core: chain directly, no gate
                                init = wp[:, si * C - 1:si * C]
                            else:
                                prev = si * C - 1
                                g = uv_p.tile([128, 1], F32, tag="g",
                                              name=f"g_{si}_{plane}_{half}")
                                nc.vector.tensor_tensor(
                                    g[:], wp[:, prev:prev + 1],
                                    gate_sb[:, si:si + 1], op=AluOp.mult)
                                init = g[:, 0:1]
                            nc.vector.tensor_tensor_scan(
                                out=wp[:, scol],
                                data0=rho_b,
                                data1=vch[:, sub * C:(sub + 1) * C],
                                initial=init,
                                op0=AluOp.mult,
                                op1=AluOp.add,
                            )
                            if half == 0 or pj == NP - 1:
                                pulse(wp[:, scol])

                for half in range(NH):
                    tcol = slice(half * R + 2 * pj * C, half * R + (2 * pj + 2) * C)
                    cs = cos_sb[:, tcol].rearrange("p (s c) -> p s c", s=2)
                    sn = sin_sb[:, tcol].rearrange("p (s c) -> p s c", s=2)
                    # s = e^{+i theta l} * w. For all but the last pair the
                    # DVE writes the cos products straight into s and the
                    # sin cross-terms ride SBUF->SBUF accumulate-DMAs (DMA
                    # data path, no DVE port cost). The last pair keeps DVE
                    # adds: its phase-D is the kernel tail and the DMA
                    # round-trip would sit on the critical path.
                    pcol = slice(2 * pj * C, (2 * pj + 2) * C)
                    wre = w_pl[0][half][:, pcol].rearrange("p (s c) -> p s c", s=2)
                    wim = w_pl[1][half][:, pcol].rearrange("p (s c) -> p s c", s=2)
                    nsn = nsin_sb[:, tcol].rearrange("p (s c) -> p s c", s=2)
                    sre = s_p.tile([128, 2 * C], BF16, tag="sch",
                                   name=f"sre_{pj}_{half}")
                    sim = s_p.tile([128, 2 * C], BF16, tag="sch",
                                   name=f"sim_{pj}_{half}")
                    if pj < NP - 1:
                        def pv2(t):
                            return t[:].rearrange("p (s c) -> p s c", s=2)
                        sre_v = pv2(sre)
                        sim_v = pv2(sim)
                        nc.vector.tensor_tensor(sre_v, wre, cs, op=AluOp.mult)
                        q2 = uv_p.tile([128, 2 * C], BF16, tag="uv", name=f"q2_{pj}_{half}")
                        nc.vector.tensor_tensor(pv2(q2), wim, nsn, op=AluOp.mult)
                        nc.gpsimd.dma_start(sre[:], q2[:], accum_op=AluOp.add)
                        nc.vector.tensor_tensor(sim_v, wim, cs, op=AluOp.mult)
                        q4 = uv_p.tile([128, 2 * C], BF16, tag="uv", name=f"q4_{pj}_{half}")
                        nc.vector.tensor_tensor(pv2(q4), wre, sn, op=AluOp.mult)
                        nc.gpsimd.dma_start(sim[:], q4[:], accum_op=AluOp.add)
                    else:
                        # last pair: per-slot DVE ops so slot 2pj's phase-D
                        # matmuls start while slot 2pj+1 is still rotating
                        for sub in range(2):
                            sl = slice(sub * C, (sub + 1) * C)
                            si = 2 * pj + sub
                            css = cos_sb[:, half * R + si * C: half * R + (si + 1) * C]
                            sns = sin_sb[:, half * R + si * C: half * R + (si + 1) * C]
                            wres = w_pl[0][half][:, si * C:(si + 1) * C]
                            wims = w_pl[1][half][:, si * C:(si + 1) * C]
                            q1 = uv_p.tile([128, C], BF16, tag="uvs", name=f"q1_{pj}_{half}_{sub}")
                            nc.vector.tensor_tensor(q1[:], wres, css, op=AluOp.mult)
                            q2 = uv_p.tile([128, C], BF16, tag="uvs", name=f"q2_{pj}_{half}_{sub}")
                            nc.vector.tensor_tensor(q2[:], wims, sns, op=AluOp.mult)
                            nc.vector.tensor_sub(sre[:, sl], q1[:], q2[:])
                            q3 = uv_p.tile([128, C], BF16, tag="uvs", name=f"q3_{pj}_{half}_{sub}")
                            nc.vector.tensor_tensor(q3[:], wims, css, op=AluOp.mult)
                            q4 = uv_p.tile([128, C], BF16, tag="uvs", name=f"q4_{pj}_{half}_{sub}")
                            nc.vector.tensor_tensor(q4[:], wres, sns, op=AluOp.mult)
                            nc.vector.tensor_add(sim[:, sl], q3[:], q4[:])
                    s_ch[0][half] = sre
                    s_ch[1][half] = sim

                pending = (pj, s_ch)

            emit_phase_d(*pending, last=True)
            warm_out = consts.tile([128, 1], F32, tag="warmout")
            nc.vector.tensor_copy(warm_out[:], warm_ps[:, 0:1])

    nc.compile()
    return nc


_NC_CACHE = {}


def _get_nc(key):
    if key not in _NC_CACHE:
        _NC_CACHE[key] = build_nc(*key)
    return _NC_CACHE[key]


# --------------------------------------------------------------------------
# host-side data prep
# --------------------------------------------------------------------------

def _host_prep(lambda_real_log, lambda_imag, log_dt, B_re, B_im, C_re, C_im):
    """Schedule-independent parameter prep: w1, w2, rho, theta."""
    lam_re = -np.exp(np.asarray(lambda_real_log, np.float64))
    lam_im = np.asarray(lambda_imag, np.float64)
    dtv = np.log1p(np.exp(np.float64(log_dt))) + 1e-4
    rho = np.exp(dtv * lam_re)                       # [N]
    theta = dtv * lam_im                             # [N]
    lam = lam_re + 1j * lam_im
    abar = np.exp(dtv * lam)
    bb = ((abar - 1.0) / lam)[:, None] * (
        np.asarray(B_re, np.float64) + 1j * np.asarray(B_im, np.float64)
    )                                                # [N, D] complex
    bb_planes = (np.ascontiguousarray(bb.real), np.ascontiguousarray(bb.imag))

    w1 = np.empty((128, DK * 2 * NH * 128), BF16_NP)
    for k in range(DK):
        for plane in range(2):
            for half in range(NH):
                col = ((k * 2 + plane) * 2 + half) * 128
                w1[:, col:col + 128] = bb_planes[plane][
                    half * 128:(half + 1) * 128, k * 128:(k + 1) * 128
                ].T.astype(np.float32)

    w2 = np.empty((128, 2 * NH * D), BF16_NP)
    c_planes = (np.asarray(C_re, np.float64), -np.asarray(C_im, np.float64))
    for plane in range(2):
        for half in range(NH):
            col = (plane * 2 + half) * D
            w2[:, col:col + D] = c_planes[plane][
                :, half * 128:(half + 1) * 128
            ].T.astype(np.float32)

    rho_in = np.empty((128, NH), np.float32)
    for half in range(NH):
        rho_in[:, half] = rho[half * 128:(half + 1) * 128]

    return w1, w2, rho_in, theta


def _pack_core(slots, x, lengths, theta, M):
    """Per-core packed inputs for one slot list."""
    R = M * C
    RT = R // 128
    xt = np.zeros((D, R), BF16_NP)
    xadd = np.zeros((R, D), BF16_NP)
    cost = np.empty((128, NH * R), BF16_NP)
    sint = np.empty((128, NH * R), BF16_NP)
    gate = np.zeros((128, M), np.float32)
    maskc = np.zeros((128, RT), np.float32)

    l_idx = np.arange(C, dtype=np.float64)
    for si, s in enumerate(slots):
        cols = slice(si * C, (si + 1) * C)
        if s.kind == "dummy":
            l0 = 0
        else:
            l0 = s.chunk * C
            xs = np.asarray(x[s.batch, l0:l0 + C, :])      # [C, D]
            xt[:, cols] = xs.T.astype(BF16_NP)
            if s.kind == "real":
                ml = np.clip(int(lengths[s.batch]) - l0, 0, C)
                rowmask = (np.arange(C) < ml).astype(np.float32)
                maskc[:, si * 4:(si + 1) * 4] = rowmask.reshape(4, 128).T
                xadd[si * C:(si + 1) * C, :] = (
                    xs * rowmask[:, None]).astype(BF16_NP)
        gate[:, si] = s.gate
        for half in range(NH):
            ph = theta[half * 128:(half + 1) * 128, None] * (l0 + l_idx)[None, :]
            tc = slice(half * R + si * C, half * R + (si + 1) * C)
            cost[:, tc] = np.cos(ph).astype(BF16_NP)
            sint[:, tc] = np.sin(ph).astype(BF16_NP)
    return {"xt": xt, "xadd": xadd, "cost": cost, "sint": sint,
            "nsint": np.ascontiguousarray(-sint), "gate": gate, "maskc": maskc}


def prepare(x, lengths, lambda_real_log, lambda_imag, log_dt,
            B_re, B_im, C_re, C_im, D_weight):
    x = np.asarray(x, np.float32)
    Dw = np.asarray(D_weight, np.float32)
    if not (Dw.shape == (D, D) and np.array_equal(Dw, np.eye(D, dtype=np.float32))):
        x_res = (x.reshape(B * L, D) @ Dw.T).reshape(B, L, D)
    else:
        x_res = x

    M, static_cont, cores = plan_schedule(np.asarray(lengths))
    w1, w2, rho_in, theta = _host_prep(
        lambda_real_log, lambda_imag, log_dt, B_re, B_im, C_re, C_im)

    in_maps = []
    for slots in cores:
        m = _pack_core(slots, x, lengths, theta, M)
        # xadd carries the masked D-term (x @ D^T); x itself feeds the SSM
        if x_res is not x:
            R = M * C
            xadd = np.zeros((R, D), BF16_NP)
            for si, s in enumerate(slots):
                if s.kind == "real":
                    l0 = s.chunk * C
                    ml = np.clip(int(lengths[s.batch]) - l0, 0, C)
                    rowmask = (np.arange(C) < ml).astype(np.float32)
                    xadd[si * C:(si + 1) * C, :] = (
                        np.asarray(x_res[s.batch, l0:l0 + C, :])
                        * rowmask[:, None]).astype(BF16_NP)
            m["xadd"] = xadd
        m.update({"w1": w1, "w2": w2, "rho": rho_in})
        in_maps.append(m)
    return (M, static_cont), cores, in_maps


def unpack_output(res, M, cores):
    y = np.zeros((B, L, D), np.float32)
    for ci, slots in enumerate(cores):
        yc = np.asarray(res.results[ci]["y"], dtype=np.float32)  # [R, D]
        for si, s in enumerate(slots):
            if s.kind == "real":
                l0 = s.chunk * C
                y[s.batch, l0:l0 + C, :] = yc[si * C:(si + 1) * C, :]
    return y


def kernel(x, lengths, lambda_real_log, lambda_imag, log_dt, B_re, B_im,
           C_re, C_im, D_weight):
    key, cores, in_maps = prepare(
        x, lengths, lambda_real_log, lambda_imag, log_dt,
        B_re, B_im, C_re, C_im, D_weight)
    M = key[0]
    nc = _get_nc(key)

    last_err = None
    for attempt in range(4):  # device errors are occasionally transient under axon
        try:
            if not _NC_CACHE.get(("warm",) + key):
                # throwaway execution: first run in a fresh process is
                # regularly ~15% slower (cold device caches / power state)
                run_bass_kernel_spmd(nc, in_maps, core_ids=list(range(NCORES)))
                _NC_CACHE[("warm",) + key] = True
            res = run_bass_kernel_spmd(nc, in_maps, core_ids=list(range(NCORES)))
            break
        except Exception as e:  # noqa: BLE001
            last_err = e
            time.sleep(5 * (attempt + 1))
    else:
        raise last_err
    return unpack_output(res, M, cores)

